# revision 27
# baseline (speedup 1.0000x reference)
"""Multi-head attention block (B=2, N=2048, C=1024, H=16, hd=64) on 8 TRN2 NeuronCores.

Sharding: data-parallel over batch (2 groups of 4 cores), tensor-parallel over
heads within each group (4 heads/core). Each core computes q/k/v for its heads,
attention, and a partial output projection; an AllGather over the 4-core group
collects head outputs, and each core projects its 256-column slice.

v2 schedule: single fused loop. The softmax exp on the Scalar engine
(~1.3us per 1024 columns x 128 iterations) and the PE matmul stream
(~164us of column-cycles) are the two near-equal rooflines, so all QKV /
projection matmuls are streamed INTO the attention loop's PE slack instead
of running in separate phases where the other engine would idle. Blocks run
pr-major ((qc,pr0) x4 then (qc,pr1) x4) so only block 0 carries forced k/v
emissions. Input DMA is a few large transfers on both HWDGE rings (SP+ACT)
ordered so the first score matmul can start ~3us in.

Per-core layouts (contraction dim on SBUF partitions; host pre-transposes x):
  xt   [1024, 2048]  x[b].T
  wqk  [1024, 512]   w_qkv columns for this core's q (256) ++ k (256)
  wv   [1024, 256]   w_qkv columns for this core's v
  wpb  [256, 1024]   w_proj rows for this core's heads (perm'd, see host code)
  bc   [128, 2]      bc[p, m] = b_proj[g*256 + m*128 + p]
  out  [256, 2048]   rows g*256:(g+1)*256 of (x[b] @ ... ).T
"""
import sys

if '/opt/trn_rl_repo' not in sys.path:
    sys.path.insert(0, '/opt/trn_rl_repo')

import numpy as np

import concourse.bass as bass
import concourse.mybir as mybir
import concourse.tile as tile
from concourse import bacc
from concourse.bass_utils import run_bass_kernel_spmd

F32 = mybir.dt.float32
F16 = mybir.dt.float16

B = 2
N = 2048          # sequence length
C = 1024          # model dim
HD = 64           # head dim
SCALE = HD ** -0.5
NT = N // 128     # 16 key tiles
CT = C // 128     # 8 contraction tiles
QC = 4            # q-chunks of 512
QCS = N // QC     # 512
GROUPS = [[0, 1, 2, 3], [4, 5, 6, 7]]
NITER = 8 * NT    # 8 blocks x 16 key tiles

_NC_CACHE = None


def _blk(b):
    """qc-major block order: b = 2*qc + pr."""
    return (b // 2, b % 2)  # (qc, pr)


def build():
    nc = bacc.Bacc(None, target_bir_lowering=False, debug=False)

    # p-major host layouts: every input DMA moves ~128 multi-KB descriptors
    # (HWDGE issue time scales with descriptor count)
    xt_ext = nc.declare_dram_parameter("xt", [QC, 128, CT, QCS], F16, isOutput=False)
    wqk_ext = nc.declare_dram_parameter("wqk", [4, 128, CT, 128], F16, isOutput=False)
    wv_ext = nc.declare_dram_parameter("wv", [128, CT, 256], F16, isOutput=False)
    wpc_ext = nc.declare_dram_parameter("wpc", [128, CT, 256], F16, isOutput=False)
    bc_ext = nc.declare_dram_parameter("bc", [128, 2], F32, isOutput=False)
    out_ext = nc.declare_dram_parameter("out", [256, N], F16, isOutput=True)

    with tile.TileContext(nc) as tc:
        with (
            tc.tile_pool(name="weights", bufs=1) as wpool,
            tc.tile_pool(name="acts", bufs=1) as apool,
            tc.tile_pool(name="expt", bufs=3) as epool,
            tc.tile_pool(name="norm", bufs=2) as npool,
            tc.tile_pool(name="outp", bufs=2) as opool,
            tc.tile_pool(name="ofp", bufs=2) as ofpool,
            tc.tile_pool(name="psS", bufs=2, space="PSUM") as psS_pool,
            tc.tile_pool(name="psE", bufs=2, space="PSUM") as psE_pool,
            tc.tile_pool(name="psO", bufs=1, space="PSUM") as psO_pool,
            tc.tile_pool(name="dramog", bufs=2, space="DRAM") as og_pool,
            tc.tile_pool(name="dramag", bufs=8, space="DRAM") as ag_pool,
        ):
            # ---- SBUF tiles ----
            # xt/wqk chunk-major so each input DMA lands in a contiguous
            # per-partition region (large descriptors)
            xt_sb = apool.tile([128, QC, CT, QCS], F16, tag="xt")
            wqk_sb = wpool.tile([128, 4, CT, 128], F16, tag="wqk")
            wv_sb = wpool.tile([128, CT, 256], F16, tag="wv")
            wp_sb = wpool.tile([128, CT, 256], F16, tag="wp")
            bc_sb = wpool.tile([128, 2], F32, tag="bc")
            ones_row = wpool.tile([1, 64], F16, tag="ones_row")
            qk_sb = apool.tile([128, 4, N], F16, tag="qk")
            v_sb = apool.tile([128, NT, 4, 128], F16, tag="v")

            # constant fills on DVE (a DMA would be thousands of descriptors);
            # v columns 65:128 are never read (PV stationary is 65 cols wide)
            nc.vector.memset(ones_row[:, :], 1.0)
            nc.vector.memset(v_sb[:, :, :, HD:HD + 1], 1.0)

            # ---- core-resync barrier: a tiny AllGather whose result feeds
            # the (table-preloading) dummy exp. The in-order ACT stream then
            # cannot start until all 4 group peers have launched, so the
            # per-block AllGathers stop paying a constant core-skew penalty.
            ones16 = npool.tile([1, 16], F16, tag="ones16")
            nc.vector.memset(ones16[:, :], 1.0)
            og_sync = og_pool.tile([1, 16], F16, tag="ogs", name="og_sync")
            nc.sync.dma_start(out=og_sync[:, :], in_=ones16[:, :])
            ag_sync = ag_pool.tile([4, 16], F16, tag="ags", name="ag_sync")
            nc.gpsimd.collective_compute(
                "AllGather", mybir.AluOpType.bypass, replica_groups=GROUPS,
                ins=[og_sync.opt()], outs=[ag_sync.opt()],
            )

            # ---- input DMAs: one per chunk, critical-path first, split over
            # both HWDGE rings. The ACT ring carries the sync-gate load plus
            # the wqk blocks (done before the first real exp needs ACT).
            # wqk blocks: 0 = q pr0, 1 = q pr1, 2 = k pr0, 3 = k pr1.
            dmy = npool.tile([1, 16], F16, tag="dmy")
            nc.scalar.dma_start(out=wqk_sb[:, 2, :, :], in_=wqk_ext.ap()[2])
            nc.scalar.dma_start(out=wqk_sb[:, 0, :, :], in_=wqk_ext.ap()[0])
            nc.scalar.dma_start(out=dmy[:, :], in_=ag_sync[0:1, :])
            nc.scalar.dma_start(out=wqk_sb[:, 1, :, :], in_=wqk_ext.ap()[1])
            nc.scalar.dma_start(out=wqk_sb[:, 3, :, :], in_=wqk_ext.ap()[3])
            nc.sync.dma_start(out=xt_sb[:, 0, :, :], in_=xt_ext.ap()[0])
            nc.sync.dma_start(out=wv_sb[:, :, :], in_=wv_ext.ap())
            nc.sync.dma_start(out=xt_sb[:, 1, :, :], in_=xt_ext.ap()[1])
            nc.sync.dma_start(out=xt_sb[:, 2, :, :], in_=xt_ext.ap()[2])
            nc.sync.dma_start(out=xt_sb[:, 3, :, :], in_=xt_ext.ap()[3])
            nc.sync.dma_start(out=wp_sb[:, :, :], in_=wpc_ext.ap())
            nc.sync.dma_start(out=bc_sb[:, :], in_=bc_ext[:, :])

            # dummy exp: preloads the Exp table AND gates ACT on the barrier
            dmy2 = npool.tile([1, 16], F16, tag="dmy2")
            nc.scalar.activation(dmy2[:, :], dmy[:, :],
                                 mybir.ActivationFunctionType.Exp,
                                 bias=0.0, scale=1.0)

            # ---- emission helpers (PE work streamed into the loop) ----
            def xtcol(ct, kt):
                off = (kt % 4) * 128
                return xt_sb[:, kt // 4, ct, off:off + 128]

            def emit_kquad(pr, nch):
                """k for 4 key tiles at once: one stationary per ct streams a
                full 512-col xt chunk, so LDWEIGHTS hides under the matmul."""
                ksl = slice(nch * QCS, (nch + 1) * QCS)
                psq = psE_pool.tile([128, QCS], F32, tag="psE",
                                    name=f"psk_{pr}_{nch}")
                for ct in range(CT):
                    nc.tensor.matmul(
                        psq[:, :],
                        wqk_sb[:, 2 + pr, ct, :],
                        xt_sb[:, nch, ct, :],
                        start=(ct == 0), stop=(ct == CT - 1),
                    )
                nc.vector.tensor_copy(qk_sb[:, 2 + pr, ksl], psq[:, :])

            def emit_khalf(pr, nch, half):
                """k for 2 key tiles (256 cols) — shortens the cold prologue."""
                lo = nch * QCS + half * 256
                psq = psE_pool.tile([128, 256], F32, tag="psE",
                                    name=f"pskh_{pr}_{nch}_{half}")
                for ct in range(CT):
                    nc.tensor.matmul(
                        psq[:, :],
                        wqk_sb[:, 2 + pr, ct, :],
                        xt_sb[:, nch, ct, half * 256:(half + 1) * 256],
                        start=(ct == 0), stop=(ct == CT - 1),
                    )
                nc.vector.tensor_copy(qk_sb[:, 2 + pr, lo:lo + 256], psq[:, :])

            def emit_q(pr, qc):
                qsl = slice(qc * QCS, (qc + 1) * QCS)
                psq = psE_pool.tile([128, QCS], F32, tag="psE", name=f"psq_{pr}_{qc}")
                for ct in range(CT):
                    nc.tensor.matmul(
                        psq[:, :],
                        wqk_sb[:, pr, ct, :],
                        xt_sb[:, qc, ct, :],
                        start=(ct == 0), stop=(ct == CT - 1),
                    )
                nc.vector.tensor_copy(qk_sb[:, pr, qsl], psq[:, :])

            def v_mms(kt):
                """The 8 accumulation matmuls for v(kt), to be interleaved
                between long attention matmuls (hides their LDWEIGHTS)."""
                psv = psE_pool.tile([128, 256], F32, tag="psE", name=f"psv_{kt}")

                def mm(ct, psv=psv, kt=kt):
                    nc.tensor.matmul(
                        psv[:, :],
                        xtcol(ct, kt),
                        wv_sb[:, ct, :],
                        start=(ct == 0), stop=(ct == CT - 1),
                    )

                def fin(psv=psv, kt=kt):
                    nc.vector.tensor_copy(
                        v_sb[:, kt, :, 0:HD],
                        psv[:, :].rearrange("p (h e) -> p h e", h=4),
                    )
                return [lambda ct=ct: mm(ct) for ct in range(CT)], fin

            def scores(b, kt):
                qc, pr = _blk(b)
                qsl = slice(qc * QCS, (qc + 1) * QCS)
                ksl = slice(kt * 128, (kt + 1) * 128)
                psS = psS_pool.tile([128, 2 * QCS], F32, tag="psS",
                                    name=f"psS_{b}_{kt}")
                nc.tensor.matmul(
                    psS[:, 0:QCS],
                    qk_sb[0:64, 2 + pr, ksl],
                    qk_sb[0:64, pr, qsl],
                    start=True, stop=True,
                )
                nc.tensor.matmul(
                    psS[:, QCS:2 * QCS],
                    qk_sb[64:128, 2 + pr, ksl],
                    qk_sb[64:128, pr, qsl],
                    start=True, stop=True,
                )
                return psS

            # ---- per-block normalize / gather / project ----
            norm_state = {}
            ags = {}

            def part1(b):
                """Drain psO: o and rowsums to SBUF (frees psO for next block)."""
                psO = norm_state.pop(('psO', b))
                o2 = npool.tile([128, QCS], F32, tag="o2", name=f"o2_{b}")
                rs_e = npool.tile([1, QCS], F32, tag="rs_e", name=f"rse_{b}")
                rs_o = npool.tile([1, QCS], F32, tag="rs_o", name=f"rso_{b}")
                # bank A (head e) first so next block's first PV can start early
                nc.vector.tensor_copy(o2[0:64, :], psO[0:64, 0:QCS])
                nc.vector.tensor_copy(rs_e[:, :], psO[64:65, 0:QCS])
                nc.vector.tensor_copy(o2[64:128, :], psO[0:64, QCS:2 * QCS])
                nc.vector.tensor_copy(rs_o[:, :], psO[64:65, QCS:2 * QCS])
                norm_state[('o2', b)] = o2
                norm_state[('rs', b)] = (rs_e, rs_o)

            def part2(b):
                """1/rowsum (fast approx), broadcast via PE matmul, normalize."""
                o2 = norm_state.pop(('o2', b))
                rs_e, rs_o = norm_state.pop(('rs', b))
                # psB borrows a psS slot (psE may be held by split proj tiles)
                psB = psS_pool.tile([128, QCS], F32, tag="psS", name=f"psB_{b}")
                for hh, rs in ((0, rs_e), (1, rs_o)):
                    rcf = npool.tile([1, QCS], F32, tag="rcf", name=f"rcf_{b}_{hh}")
                    nc.vector.reciprocal_approx_fast(out=rcf[:, :], in_=rs[:, :])
                    rc16 = npool.tile([1, QCS], F16, tag="rc16", name=f"rc16_{b}_{hh}")
                    nc.vector.tensor_copy(rc16[:, :], rcf[:, :])
                    nc.tensor.matmul(psB[hh * 64:(hh + 1) * 64, :],
                                     ones_row[:, :], rc16[:, :],
                                     start=True, stop=True)
                on_sb = npool.tile([128, QCS], F16, tag="on", name=f"on_{b}")
                nc.vector.tensor_mul(on_sb[:, :], o2[:, :], psB[:, :])
                norm_state[('on', b)] = on_sb

            def part3(b):
                """Store + AllGather this block's head outputs."""
                qc, pr = _blk(b)
                on_sb = norm_state.pop(('on', b))
                og = og_pool.tile([128, QCS], F16, tag="og", name=f"og_{b}")
                nc.sync.dma_start(out=og[:, :], in_=on_sb[:, :])
                ag = ag_pool.tile([512, QCS], F16, tag="ag", name=f"ag_{b}")
                nc.gpsimd.collective_compute(
                    "AllGather",
                    mybir.AluOpType.bypass,
                    replica_groups=GROUPS,
                    ins=[og.opt()],
                    outs=[ag.opt()],
                )
                ags[(qc, pr)] = ag

            of_sbs = {}
            proj_ps = {}

            def emit_ofload(qc, half=None):
                if qc in of_sbs:
                    of_sb = of_sbs[qc]
                else:
                    of_sb = ofpool.tile([128, CT, QCS], F16, tag="of",
                                        name=f"of_{qc}")
                    of_sbs[qc] = of_sb
                prs = range(2) if half is None else [half]
                for pr in prs:
                    ag_r = ags[(qc, pr)][:, :].rearrange("(t p) n -> p t n", p=128)
                    nc.sync.dma_start(out=of_sb[:, pr * 4:(pr + 1) * 4, :], in_=ag_r)

            def emit_proj(qc, m2, lo=0, hi=CT):
                qsl = slice(qc * QCS, (qc + 1) * QCS)
                of_sb = of_sbs[qc]
                if (qc, m2) in proj_ps:
                    psP = proj_ps[(qc, m2)]
                else:
                    psP = psE_pool.tile([128, QCS], F32, tag="psE",
                                        name=f"psP_{qc}_{m2}")
                    proj_ps[(qc, m2)] = psP
                for t in range(lo, hi):
                    nc.tensor.matmul(
                        psP[:, :],
                        wp_sb[:, t, m2 * 128:(m2 + 1) * 128],
                        of_sb[:, t, :],
                        start=(t == 0), stop=(t == CT - 1),
                    )
                if hi == CT:
                    outsb = opool.tile([128, QCS], F16, tag="outsb",
                                       name=f"outsb_{qc}_{m2}")
                    nc.vector.tensor_scalar_add(outsb[:, :], psP[:, :],
                                                bc_sb[:, m2:m2 + 1])
                    nc.sync.dma_start(out=out_ext[m2 * 128:(m2 + 1) * 128, qsl],
                                      in_=outsb[:, :])

            # ---- static emission schedule: iter -> list of thunks ----
            sched = {}

            def at(i, fn):
                sched.setdefault(i, []).append(fn)

            at(0, lambda: emit_khalf(0, 0, 1))    # k(pr0) kt2-3, due iter 1
            at(1, lambda: emit_kquad(0, 1))       # k(pr0) due iters 4/8/12
            at(5, lambda: emit_kquad(0, 2))
            at(9, lambda: emit_kquad(0, 3))
            at(11, lambda: emit_q(1, 0))          # due block 1 (iter 15)
            at(12, lambda: emit_kquad(1, 0))      # k(pr1) due iter 16/20/24/28
            at(17, lambda: emit_kquad(1, 1))
            at(21, lambda: emit_kquad(1, 2))
            at(25, lambda: emit_kquad(1, 3))
            at(27, lambda: emit_q(0, 1))          # due block 2 (iter 31)
            at(43, lambda: emit_q(1, 1))          # due block 3 (iter 47)
            at(59, lambda: emit_q(0, 2))          # due block 4 (iter 63)
            at(75, lambda: emit_q(1, 2))          # due block 5 (iter 79)
            at(91, lambda: emit_q(0, 3))          # due block 6 (iter 95)
            at(107, lambda: emit_q(1, 3))         # due block 7 (iter 111)
            for qc in range(3):                   # proj(qc): AG(qc,1) done at
                at(32 * qc + 49, lambda qc=qc: emit_ofload(qc))   # ~iter 38+32qc
                at(32 * qc + 52, lambda qc=qc: emit_proj(qc, 0))
                at(32 * qc + 54, lambda qc=qc: emit_proj(qc, 1))
            # proj(3): pr0-half of the contraction runs before the last AG
            at(120, lambda: emit_ofload(3, half=0))
            at(122, lambda: emit_proj(3, 0, 0, 4))
            at(124, lambda: emit_proj(3, 1, 0, 4))
            for b in range(7):                    # normalize tail of each block
                at(16 * b + 17, lambda b=b: part2(b))
                at(16 * b + 18, lambda b=b: part3(b))

            # v(kt) streamed during block 0, matmuls interleaved between the
            # long attention matmuls so each LDWEIGHTS hides under them
            v_plan = {kt - 1: kt for kt in range(1, NT)}

            # ---- prologue (kept short: it runs at the cold PE clock) ----
            emit_khalf(0, 0, 0)
            emit_q(0, 0)
            vm, vfin = v_mms(0)
            for m in vm:
                m()
            vfin()
            psS_cur = scores(0, 0)

            # ---- main loop ----
            for bi in range(NITER):
                b, kt = bi // NT, bi % NT
                qc, pr = _blk(b)
                vkt = v_plan.get(bi)
                vm, vfin = v_mms(vkt) if vkt is not None else ([], None)
                if bi + 1 < NITER:
                    nb, nkt = (b, kt + 1) if kt < NT - 1 else (b + 1, 0)
                    nqc, npr = _blk(nb)
                    qsl_ = slice(nqc * QCS, (nqc + 1) * QCS)
                    ksl_ = slice(nkt * 128, (nkt + 1) * 128)
                    psS_next = psS_pool.tile([128, 2 * QCS], F32, tag="psS",
                                             name=f"psS_{nb}_{nkt}")
                    nc.tensor.matmul(psS_next[:, 0:QCS],
                                     qk_sb[0:64, 2 + npr, ksl_],
                                     qk_sb[0:64, npr, qsl_],
                                     start=True, stop=True)
                    for m in vm[0:2]:
                        m()
                    nc.tensor.matmul(psS_next[:, QCS:2 * QCS],
                                     qk_sb[64:128, 2 + npr, ksl_],
                                     qk_sb[64:128, npr, qsl_],
                                     start=True, stop=True)
                    for m in vm[2:4]:
                        m()
                else:
                    psS_next = None
                    for m in vm[0:4]:
                        m()
                expt = epool.tile([128, 2 * QCS], F16, tag="expt",
                                  name=f"expt_{bi}")
                nc.scalar.activation(
                    expt[:, :], psS_cur[:, :],
                    mybir.ActivationFunctionType.Exp,
                    bias=0.0, scale=SCALE,
                )
                for fn in sched.get(bi, ()):
                    fn()
                if kt == 0:
                    norm_state[('psO', b)] = psO_pool.tile(
                        [128, 2 * QCS], F32, tag="psO", name=f"psO_{b}")
                psO = norm_state[('psO', b)]
                nc.tensor.matmul(
                    psO[0:65, 0:QCS],
                    v_sb[:, kt, 2 * pr, 0:65],
                    expt[:, 0:QCS],
                    start=(kt == 0), stop=(kt == NT - 1),
                )
                for m in vm[4:6]:
                    m()
                nc.tensor.matmul(
                    psO[0:65, QCS:2 * QCS],
                    v_sb[:, kt, 2 * pr + 1, 0:65],
                    expt[:, QCS:2 * QCS],
                    start=(kt == 0), stop=(kt == NT - 1),
                )
                for m in vm[6:8]:
                    m()
                if vfin is not None:
                    vfin()
                psS_cur = psS_next
                if kt == NT - 1:
                    part1(b)

            # ---- tail: last block's normalize + gather + projection ----
            part2(7)
            part3(7)
            emit_ofload(3, half=1)
            emit_proj(3, 0, 4, CT)
            emit_proj(3, 1, 4, CT)

    nc.compile()
    return nc


def _get_nc():
    global _NC_CACHE
    if _NC_CACHE is None:
        _NC_CACHE = build()
    return _NC_CACHE


def shard_inputs(x, w_qkv, w_proj, b_proj):
    x = np.asarray(x, dtype=np.float32)
    w_qkv = np.asarray(w_qkv, dtype=np.float32)
    w_proj = np.asarray(w_proj, dtype=np.float32)
    b_proj = np.asarray(b_proj, dtype=np.float32)
    # ag row order: for each pr, rank-major then local-head-major:
    # rows [j*128 + h2*64 + e] <-> global head 4j + 2*pr + h2
    perm = np.concatenate([
        np.arange(1024).reshape(16, 64)[[4 * j + 2 * pr + h2 for j in range(4) for h2 in range(2)]].reshape(-1)
        for pr in range(2)
    ])
    def pmajor_kt(w):
        # [C, M] -> [128(p), CT(t), M]: row t*128+p -> [p, t]
        return np.ascontiguousarray(
            w.reshape(CT, 128, w.shape[1]).transpose(1, 0, 2).astype(np.float16))

    in_maps = []
    for core in range(8):
        b, g = divmod(core, 4)
        cs = slice(g * 256, (g + 1) * 256)
        xtT = x[b].T  # [C, N]
        # xt: [QC(nch), 128(p), CT(t), QCS] with [nch,p,t,c] = xtT[t*128+p, nch*512+c]
        xt_arr = np.ascontiguousarray(
            xtT.reshape(CT, 128, QC, QCS).transpose(2, 1, 0, 3).astype(np.float16))
        # wqk blocks: 0 = q pr0, 1 = q pr1, 2 = k pr0, 3 = k pr1 (128 cols each)
        qcols = w_qkv[:, 0 * C + g * 256:0 * C + (g + 1) * 256]
        kcols = w_qkv[:, 1 * C + g * 256:1 * C + (g + 1) * 256]
        wqk = np.concatenate([qcols, kcols], axis=1)  # [C, 512]
        wqk_arr = np.ascontiguousarray(
            wqk.reshape(CT, 128, 4, 128).transpose(2, 1, 0, 3).astype(np.float16))
        in_maps.append({
            "xt": xt_arr,
            "wqk": wqk_arr,
            "wv": pmajor_kt(w_qkv[:, 2 * C + g * 256:2 * C + (g + 1) * 256]),
            "wpc": pmajor_kt(w_proj[perm, :][:, cs]),
            "bc": np.ascontiguousarray(b_proj[cs].reshape(2, 128).T),
        })
    return in_maps


def assemble_output(results):
    outT = np.empty((B, C, N), dtype=np.float32)
    for core in range(8):
        b, g = divmod(core, 4)
        outT[b, g * 256:(g + 1) * 256, :] = np.asarray(results[core]["out"], dtype=np.float32)
    return np.ascontiguousarray(outT.transpose(0, 2, 1))


def run_sharded(x, w_qkv, w_proj, b_proj, trace=False):
    nc = _get_nc()
    in_maps = shard_inputs(x, w_qkv, w_proj, b_proj)
    res = run_bass_kernel_spmd(nc, in_maps, core_ids=list(range(8)), trace=trace)
    return assemble_output(res.results), res.exec_time_ns


def kernel(x, w_qkv, w_proj, b_proj):
    out, _ = run_sharded(x, w_qkv, w_proj, b_proj, trace=False)
    return out


# revision 28
# speedup vs baseline: 1.0620x; 1.0620x over previous
"""Multi-head attention block (B=2, N=2048, C=1024, H=16, hd=64) on 8 TRN2 NeuronCores.

Sharding: data-parallel over batch (2 groups of 4 cores), tensor-parallel over
heads within each group (4 heads/core). Each core computes q/k/v for its heads,
attention, and a partial output projection; an AllGather over the 4-core group
collects head outputs, and each core projects its 256-column slice.

v2 schedule: single fused loop. The softmax exp on the Scalar engine
(~1.3us per 1024 columns x 128 iterations) and the PE matmul stream
(~164us of column-cycles) are the two near-equal rooflines, so all QKV /
projection matmuls are streamed INTO the attention loop's PE slack instead
of running in separate phases where the other engine would idle. Blocks run
pr-major ((qc,pr0) x4 then (qc,pr1) x4) so only block 0 carries forced k/v
emissions. Input DMA is a few large transfers on both HWDGE rings (SP+ACT)
ordered so the first score matmul can start ~3us in.

Per-core layouts (contraction dim on SBUF partitions; host pre-transposes x):
  xt   [1024, 2048]  x[b].T
  wqk  [1024, 512]   w_qkv columns for this core's q (256) ++ k (256)
  wv   [1024, 256]   w_qkv columns for this core's v
  wpb  [256, 1024]   w_proj rows for this core's heads (perm'd, see host code)
  bc   [128, 2]      bc[p, m] = b_proj[g*256 + m*128 + p]
  out  [256, 2048]   rows g*256:(g+1)*256 of (x[b] @ ... ).T
"""
import sys

if '/opt/trn_rl_repo' not in sys.path:
    sys.path.insert(0, '/opt/trn_rl_repo')

import numpy as np

import concourse.bass as bass
import concourse.mybir as mybir
import concourse.tile as tile
from concourse import bacc
from concourse.bass_utils import run_bass_kernel_spmd

F32 = mybir.dt.float32
F16 = mybir.dt.float16

B = 2
N = 2048          # sequence length
C = 1024          # model dim
HD = 64           # head dim
SCALE = HD ** -0.5
NT = N // 128     # 16 key tiles
CT = C // 128     # 8 contraction tiles
QC = 4            # q-chunks of 512
QCS = N // QC     # 512
GROUPS = [[0, 1, 2, 3], [4, 5, 6, 7]]
NITER = 8 * NT    # 8 blocks x 16 key tiles

_NC_CACHE = None


def _blk(b):
    """qc-major block order: b = 2*qc + pr."""
    return (b // 2, b % 2)  # (qc, pr)


def build():
    nc = bacc.Bacc(None, target_bir_lowering=False, debug=False)

    # p-major host layouts: every input DMA moves ~128 multi-KB descriptors
    # (HWDGE issue time scales with descriptor count)
    xt_ext = nc.declare_dram_parameter("xt", [QC, 128, CT, QCS], F16, isOutput=False)
    wqk_ext = nc.declare_dram_parameter("wqk", [4, 128, CT, 128], F16, isOutput=False)
    wv_ext = nc.declare_dram_parameter("wv", [128, CT, 256], F16, isOutput=False)
    wpc_ext = nc.declare_dram_parameter("wpc", [128, CT, 256], F16, isOutput=False)
    bc_ext = nc.declare_dram_parameter("bc", [128, 2], F32, isOutput=False)
    out_ext = nc.declare_dram_parameter("out", [256, N], F16, isOutput=True)

    with tile.TileContext(nc) as tc:
        with (
            tc.tile_pool(name="weights", bufs=1) as wpool,
            tc.tile_pool(name="acts", bufs=1) as apool,
            tc.tile_pool(name="expt", bufs=3) as epool,
            tc.tile_pool(name="norm", bufs=2) as npool,
            tc.tile_pool(name="outp", bufs=2) as opool,
            tc.tile_pool(name="ofp", bufs=2) as ofpool,
            tc.tile_pool(name="psS", bufs=2, space="PSUM") as psS_pool,
            tc.tile_pool(name="psE", bufs=2, space="PSUM") as psE_pool,
            tc.tile_pool(name="psO", bufs=1, space="PSUM") as psO_pool,
            tc.tile_pool(name="dramog", bufs=2, space="DRAM") as og_pool,
            tc.tile_pool(name="dramag", bufs=8, space="DRAM") as ag_pool,
        ):
            # ---- SBUF tiles ----
            # xt/wqk chunk-major so each input DMA lands in a contiguous
            # per-partition region (large descriptors)
            xt_sb = apool.tile([128, QC, CT, QCS], F16, tag="xt")
            wqk_sb = wpool.tile([128, 4, CT, 128], F16, tag="wqk")
            wv_sb = wpool.tile([128, CT, 256], F16, tag="wv")
            wp_sb = wpool.tile([128, CT, 256], F16, tag="wp")
            bc_sb = wpool.tile([128, 2], F32, tag="bc")
            ones_row = wpool.tile([1, 64], F16, tag="ones_row")
            qk_sb = apool.tile([128, 4, N], F16, tag="qk")
            v_sb = apool.tile([128, NT, 4, 128], F16, tag="v")

            # constant fills on DVE (a DMA would be thousands of descriptors);
            # v columns 65:128 are never read (PV stationary is 65 cols wide)
            nc.vector.memset(ones_row[:, :], 1.0)
            nc.vector.memset(v_sb[:, :, :, HD:HD + 1], 1.0)

            # preload the Exp activation table while input DMAs stream
            dmy = npool.tile([1, 16], F16, tag="dmy")
            nc.vector.memset(dmy[:, :], 0.0)
            dmy2 = npool.tile([1, 16], F16, tag="dmy2")
            nc.scalar.activation(dmy2[:, :], dmy[:, :],
                                 mybir.ActivationFunctionType.Exp,
                                 bias=0.0, scale=1.0)

            # ---- input DMAs: one per chunk, critical-path first, split over
            # both HWDGE rings (the wqk blocks finish on the ACT ring before
            # the first real exp needs the ACT sequencer).
            # wqk blocks: 0 = q pr0, 1 = q pr1, 2 = k pr0, 3 = k pr1.
            nc.scalar.dma_start(out=wqk_sb[:, 2, :, :], in_=wqk_ext.ap()[2])
            nc.scalar.dma_start(out=wqk_sb[:, 0, :, :], in_=wqk_ext.ap()[0])
            nc.scalar.dma_start(out=wqk_sb[:, 1, :, :], in_=wqk_ext.ap()[1])
            nc.scalar.dma_start(out=wqk_sb[:, 3, :, :], in_=wqk_ext.ap()[3])
            nc.sync.dma_start(out=xt_sb[:, 0, :, :], in_=xt_ext.ap()[0])
            nc.sync.dma_start(out=wv_sb[:, :, :], in_=wv_ext.ap())
            nc.sync.dma_start(out=xt_sb[:, 1, :, :], in_=xt_ext.ap()[1])
            nc.sync.dma_start(out=xt_sb[:, 2, :, :], in_=xt_ext.ap()[2])
            nc.sync.dma_start(out=xt_sb[:, 3, :, :], in_=xt_ext.ap()[3])
            nc.sync.dma_start(out=wp_sb[:, :, :], in_=wpc_ext.ap())
            nc.sync.dma_start(out=bc_sb[:, :], in_=bc_ext[:, :])

            # ---- emission helpers (PE work streamed into the loop) ----
            def xtcol(ct, kt):
                off = (kt % 4) * 128
                return xt_sb[:, kt // 4, ct, off:off + 128]

            def emit_kquad(pr, nch):
                """k for 4 key tiles at once: one stationary per ct streams a
                full 512-col xt chunk, so LDWEIGHTS hides under the matmul."""
                ksl = slice(nch * QCS, (nch + 1) * QCS)
                psq = psE_pool.tile([128, QCS], F32, tag="psE",
                                    name=f"psk_{pr}_{nch}")
                for ct in range(CT):
                    nc.tensor.matmul(
                        psq[:, :],
                        wqk_sb[:, 2 + pr, ct, :],
                        xt_sb[:, nch, ct, :],
                        start=(ct == 0), stop=(ct == CT - 1),
                    )
                nc.vector.tensor_copy(qk_sb[:, 2 + pr, ksl], psq[:, :])

            def emit_khalf(pr, nch, half):
                """k for 2 key tiles (256 cols) — shortens the cold prologue."""
                lo = nch * QCS + half * 256
                psq = psE_pool.tile([128, 256], F32, tag="psE",
                                    name=f"pskh_{pr}_{nch}_{half}")
                for ct in range(CT):
                    nc.tensor.matmul(
                        psq[:, :],
                        wqk_sb[:, 2 + pr, ct, :],
                        xt_sb[:, nch, ct, half * 256:(half + 1) * 256],
                        start=(ct == 0), stop=(ct == CT - 1),
                    )
                nc.vector.tensor_copy(qk_sb[:, 2 + pr, lo:lo + 256], psq[:, :])

            def emit_q(pr, qc):
                qsl = slice(qc * QCS, (qc + 1) * QCS)
                psq = psE_pool.tile([128, QCS], F32, tag="psE", name=f"psq_{pr}_{qc}")
                for ct in range(CT):
                    nc.tensor.matmul(
                        psq[:, :],
                        wqk_sb[:, pr, ct, :],
                        xt_sb[:, qc, ct, :],
                        start=(ct == 0), stop=(ct == CT - 1),
                    )
                nc.vector.tensor_copy(qk_sb[:, pr, qsl], psq[:, :])

            def v_mms(kt):
                """The 8 accumulation matmuls for v(kt), to be interleaved
                between long attention matmuls (hides their LDWEIGHTS)."""
                psv = psE_pool.tile([128, 256], F32, tag="psE", name=f"psv_{kt}")

                def mm(ct, psv=psv, kt=kt):
                    nc.tensor.matmul(
                        psv[:, :],
                        xtcol(ct, kt),
                        wv_sb[:, ct, :],
                        start=(ct == 0), stop=(ct == CT - 1),
                    )

                def fin(psv=psv, kt=kt):
                    nc.vector.tensor_copy(
                        v_sb[:, kt, :, 0:HD],
                        psv[:, :].rearrange("p (h e) -> p h e", h=4),
                    )
                return [lambda ct=ct: mm(ct) for ct in range(CT)], fin

            def scores(b, kt):
                qc, pr = _blk(b)
                qsl = slice(qc * QCS, (qc + 1) * QCS)
                ksl = slice(kt * 128, (kt + 1) * 128)
                psS = psS_pool.tile([128, 2 * QCS], F32, tag="psS",
                                    name=f"psS_{b}_{kt}")
                nc.tensor.matmul(
                    psS[:, 0:QCS],
                    qk_sb[0:64, 2 + pr, ksl],
                    qk_sb[0:64, pr, qsl],
                    start=True, stop=True,
                )
                nc.tensor.matmul(
                    psS[:, QCS:2 * QCS],
                    qk_sb[64:128, 2 + pr, ksl],
                    qk_sb[64:128, pr, qsl],
                    start=True, stop=True,
                )
                return psS

            # ---- per-block normalize / gather / project ----
            norm_state = {}
            ags = {}

            def part1(b):
                """Drain psO: o and rowsums to SBUF (frees psO for next block)."""
                psO = norm_state.pop(('psO', b))
                o2 = npool.tile([128, QCS], F32, tag="o2", name=f"o2_{b}")
                rs_e = npool.tile([1, QCS], F32, tag="rs_e", name=f"rse_{b}")
                rs_o = npool.tile([1, QCS], F32, tag="rs_o", name=f"rso_{b}")
                # bank A (head e) first so next block's first PV can start early
                nc.vector.tensor_copy(o2[0:64, :], psO[0:64, 0:QCS])
                nc.vector.tensor_copy(rs_e[:, :], psO[64:65, 0:QCS])
                nc.vector.tensor_copy(o2[64:128, :], psO[0:64, QCS:2 * QCS])
                nc.vector.tensor_copy(rs_o[:, :], psO[64:65, QCS:2 * QCS])
                norm_state[('o2', b)] = o2
                norm_state[('rs', b)] = (rs_e, rs_o)

            def part2(b):
                """1/rowsum (fast approx), broadcast via PE matmul, normalize."""
                o2 = norm_state.pop(('o2', b))
                rs_e, rs_o = norm_state.pop(('rs', b))
                # psB borrows a psS slot (psE may be held by split proj tiles)
                psB = psS_pool.tile([128, QCS], F32, tag="psS", name=f"psB_{b}")
                for hh, rs in ((0, rs_e), (1, rs_o)):
                    rcf = npool.tile([1, QCS], F32, tag="rcf", name=f"rcf_{b}_{hh}")
                    nc.vector.reciprocal_approx_fast(out=rcf[:, :], in_=rs[:, :])
                    rc16 = npool.tile([1, QCS], F16, tag="rc16", name=f"rc16_{b}_{hh}")
                    nc.vector.tensor_copy(rc16[:, :], rcf[:, :])
                    nc.tensor.matmul(psB[hh * 64:(hh + 1) * 64, :],
                                     ones_row[:, :], rc16[:, :],
                                     start=True, stop=True)
                on_sb = npool.tile([128, QCS], F16, tag="on", name=f"on_{b}")
                nc.vector.tensor_mul(on_sb[:, :], o2[:, :], psB[:, :])
                norm_state[('on', b)] = on_sb

            def part3(b):
                """Store + AllGather this block's head outputs."""
                qc, pr = _blk(b)
                on_sb = norm_state.pop(('on', b))
                og = og_pool.tile([128, QCS], F16, tag="og", name=f"og_{b}")
                nc.sync.dma_start(out=og[:, :], in_=on_sb[:, :])
                ag = ag_pool.tile([512, QCS], F16, tag="ag", name=f"ag_{b}")
                nc.gpsimd.collective_compute(
                    "AllGather",
                    mybir.AluOpType.bypass,
                    replica_groups=GROUPS,
                    ins=[og.opt()],
                    outs=[ag.opt()],
                )
                ags[(qc, pr)] = ag

            of_sbs = {}
            proj_ps = {}

            def emit_ofload(qc, half=None):
                if qc in of_sbs:
                    of_sb = of_sbs[qc]
                else:
                    of_sb = ofpool.tile([128, CT, QCS], F16, tag="of",
                                        name=f"of_{qc}")
                    of_sbs[qc] = of_sb
                prs = range(2) if half is None else [half]
                for pr in prs:
                    ag_r = ags[(qc, pr)][:, :].rearrange("(t p) n -> p t n", p=128)
                    nc.sync.dma_start(out=of_sb[:, pr * 4:(pr + 1) * 4, :], in_=ag_r)

            def emit_proj(qc, m2, lo=0, hi=CT):
                qsl = slice(qc * QCS, (qc + 1) * QCS)
                of_sb = of_sbs[qc]
                if (qc, m2) in proj_ps:
                    psP = proj_ps[(qc, m2)]
                else:
                    psP = psE_pool.tile([128, QCS], F32, tag="psE",
                                        name=f"psP_{qc}_{m2}")
                    proj_ps[(qc, m2)] = psP
                for t in range(lo, hi):
                    nc.tensor.matmul(
                        psP[:, :],
                        wp_sb[:, t, m2 * 128:(m2 + 1) * 128],
                        of_sb[:, t, :],
                        start=(t == 0), stop=(t == CT - 1),
                    )
                if hi == CT:
                    outsb = opool.tile([128, QCS], F16, tag="outsb",
                                       name=f"outsb_{qc}_{m2}")
                    nc.vector.tensor_scalar_add(outsb[:, :], psP[:, :],
                                                bc_sb[:, m2:m2 + 1])
                    nc.sync.dma_start(out=out_ext[m2 * 128:(m2 + 1) * 128, qsl],
                                      in_=outsb[:, :])

            # ---- static emission schedule: iter -> list of thunks ----
            sched = {}

            def at(i, fn):
                sched.setdefault(i, []).append(fn)

            at(0, lambda: emit_khalf(0, 0, 1))    # k(pr0) kt2-3, due iter 1
            at(1, lambda: emit_kquad(0, 1))       # k(pr0) due iters 4/8/12
            at(5, lambda: emit_kquad(0, 2))
            at(9, lambda: emit_kquad(0, 3))
            at(11, lambda: emit_q(1, 0))          # due block 1 (iter 15)
            at(12, lambda: emit_kquad(1, 0))      # k(pr1) due iter 16/20/24/28
            at(17, lambda: emit_kquad(1, 1))
            at(21, lambda: emit_kquad(1, 2))
            at(25, lambda: emit_kquad(1, 3))
            at(27, lambda: emit_q(0, 1))          # due block 2 (iter 31)
            at(43, lambda: emit_q(1, 1))          # due block 3 (iter 47)
            at(59, lambda: emit_q(0, 2))          # due block 4 (iter 63)
            at(75, lambda: emit_q(1, 2))          # due block 5 (iter 79)
            at(91, lambda: emit_q(0, 3))          # due block 6 (iter 95)
            at(107, lambda: emit_q(1, 3))         # due block 7 (iter 111)
            for qc in range(3):                   # proj(qc): AG(qc,1) done at
                at(32 * qc + 49, lambda qc=qc: emit_ofload(qc))   # ~iter 38+32qc
                at(32 * qc + 52, lambda qc=qc: emit_proj(qc, 0))
                at(32 * qc + 54, lambda qc=qc: emit_proj(qc, 1))
            # proj(3): pr0-half of the contraction runs before the last AG
            at(120, lambda: emit_ofload(3, half=0))
            at(122, lambda: emit_proj(3, 0, 0, 4))
            at(124, lambda: emit_proj(3, 1, 0, 4))
            for b in range(7):                    # normalize tail of each block
                at(16 * b + 17, lambda b=b: part2(b))
                at(16 * b + 18, lambda b=b: part3(b))

            # v(kt) streamed during block 0, matmuls interleaved between the
            # long attention matmuls so each LDWEIGHTS hides under them
            v_plan = {kt - 1: kt for kt in range(1, NT)}

            # ---- prologue (kept short: it runs at the cold PE clock) ----
            emit_khalf(0, 0, 0)
            emit_q(0, 0)
            vm, vfin = v_mms(0)
            for m in vm:
                m()
            vfin()
            psS_cur = scores(0, 0)

            # ---- main loop ----
            for bi in range(NITER):
                b, kt = bi // NT, bi % NT
                qc, pr = _blk(b)
                vkt = v_plan.get(bi)
                vm, vfin = v_mms(vkt) if vkt is not None else ([], None)
                if bi + 1 < NITER:
                    nb, nkt = (b, kt + 1) if kt < NT - 1 else (b + 1, 0)
                    nqc, npr = _blk(nb)
                    qsl_ = slice(nqc * QCS, (nqc + 1) * QCS)
                    ksl_ = slice(nkt * 128, (nkt + 1) * 128)
                    psS_next = psS_pool.tile([128, 2 * QCS], F32, tag="psS",
                                             name=f"psS_{nb}_{nkt}")
                    nc.tensor.matmul(psS_next[:, 0:QCS],
                                     qk_sb[0:64, 2 + npr, ksl_],
                                     qk_sb[0:64, npr, qsl_],
                                     start=True, stop=True)
                    for m in vm[0:2]:
                        m()
                    nc.tensor.matmul(psS_next[:, QCS:2 * QCS],
                                     qk_sb[64:128, 2 + npr, ksl_],
                                     qk_sb[64:128, npr, qsl_],
                                     start=True, stop=True)
                    for m in vm[2:4]:
                        m()
                else:
                    psS_next = None
                    for m in vm[0:4]:
                        m()
                expt = epool.tile([128, 2 * QCS], F16, tag="expt",
                                  name=f"expt_{bi}")
                nc.scalar.activation(
                    expt[:, :], psS_cur[:, :],
                    mybir.ActivationFunctionType.Exp,
                    bias=0.0, scale=SCALE,
                )
                for fn in sched.get(bi, ()):
                    fn()
                if kt == 0:
                    norm_state[('psO', b)] = psO_pool.tile(
                        [128, 2 * QCS], F32, tag="psO", name=f"psO_{b}")
                psO = norm_state[('psO', b)]
                nc.tensor.matmul(
                    psO[0:65, 0:QCS],
                    v_sb[:, kt, 2 * pr, 0:65],
                    expt[:, 0:QCS],
                    start=(kt == 0), stop=(kt == NT - 1),
                )
                for m in vm[4:6]:
                    m()
                nc.tensor.matmul(
                    psO[0:65, QCS:2 * QCS],
                    v_sb[:, kt, 2 * pr + 1, 0:65],
                    expt[:, QCS:2 * QCS],
                    start=(kt == 0), stop=(kt == NT - 1),
                )
                for m in vm[6:8]:
                    m()
                if vfin is not None:
                    vfin()
                psS_cur = psS_next
                if kt == NT - 1:
                    part1(b)

            # ---- tail: last block's normalize + gather + projection ----
            part2(7)
            part3(7)
            emit_ofload(3, half=1)
            emit_proj(3, 0, 4, CT)
            emit_proj(3, 1, 4, CT)

    nc.compile()
    return nc


def _get_nc():
    global _NC_CACHE
    if _NC_CACHE is None:
        _NC_CACHE = build()
    return _NC_CACHE


def shard_inputs(x, w_qkv, w_proj, b_proj):
    x = np.asarray(x, dtype=np.float32)
    w_qkv = np.asarray(w_qkv, dtype=np.float32)
    w_proj = np.asarray(w_proj, dtype=np.float32)
    b_proj = np.asarray(b_proj, dtype=np.float32)
    # ag row order: for each pr, rank-major then local-head-major:
    # rows [j*128 + h2*64 + e] <-> global head 4j + 2*pr + h2
    perm = np.concatenate([
        np.arange(1024).reshape(16, 64)[[4 * j + 2 * pr + h2 for j in range(4) for h2 in range(2)]].reshape(-1)
        for pr in range(2)
    ])
    def pmajor_kt(w):
        # [C, M] -> [128(p), CT(t), M]: row t*128+p -> [p, t]
        return np.ascontiguousarray(
            w.reshape(CT, 128, w.shape[1]).transpose(1, 0, 2).astype(np.float16))

    in_maps = []
    for core in range(8):
        b, g = divmod(core, 4)
        cs = slice(g * 256, (g + 1) * 256)
        xtT = x[b].T  # [C, N]
        # xt: [QC(nch), 128(p), CT(t), QCS] with [nch,p,t,c] = xtT[t*128+p, nch*512+c]
        xt_arr = np.ascontiguousarray(
            xtT.reshape(CT, 128, QC, QCS).transpose(2, 1, 0, 3).astype(np.float16))
        # wqk blocks: 0 = q pr0, 1 = q pr1, 2 = k pr0, 3 = k pr1 (128 cols each)
        qcols = w_qkv[:, 0 * C + g * 256:0 * C + (g + 1) * 256]
        kcols = w_qkv[:, 1 * C + g * 256:1 * C + (g + 1) * 256]
        wqk = np.concatenate([qcols, kcols], axis=1)  # [C, 512]
        wqk_arr = np.ascontiguousarray(
            wqk.reshape(CT, 128, 4, 128).transpose(2, 1, 0, 3).astype(np.float16))
        in_maps.append({
            "xt": xt_arr,
            "wqk": wqk_arr,
            "wv": pmajor_kt(w_qkv[:, 2 * C + g * 256:2 * C + (g + 1) * 256]),
            "wpc": pmajor_kt(w_proj[perm, :][:, cs]),
            "bc": np.ascontiguousarray(b_proj[cs].reshape(2, 128).T),
        })
    return in_maps


def assemble_output(results):
    outT = np.empty((B, C, N), dtype=np.float32)
    for core in range(8):
        b, g = divmod(core, 4)
        outT[b, g * 256:(g + 1) * 256, :] = np.asarray(results[core]["out"], dtype=np.float32)
    return np.ascontiguousarray(outT.transpose(0, 2, 1))


def run_sharded(x, w_qkv, w_proj, b_proj, trace=False):
    nc = _get_nc()
    in_maps = shard_inputs(x, w_qkv, w_proj, b_proj)
    res = run_bass_kernel_spmd(nc, in_maps, core_ids=list(range(8)), trace=trace)
    return assemble_output(res.results), res.exec_time_ns


def kernel(x, w_qkv, w_proj, b_proj):
    out, _ = run_sharded(x, w_qkv, w_proj, b_proj, trace=False)
    return out


# revision 30
# speedup vs baseline: 1.1547x; 1.0873x over previous
"""Multi-head attention block (B=2, N=2048, C=1024, H=16, hd=64) on 8 TRN2 NeuronCores.

Sharding: data-parallel over batch (2 groups of 4 cores), tensor-parallel over
heads within each group (4 heads/core). Each core computes q/k/v for its heads,
attention, and a partial output projection; an AllGather over the 4-core group
collects head outputs, and each core projects its 256-column slice.

v2 schedule: single fused loop. The softmax exp on the Scalar engine
(~1.3us per 1024 columns x 128 iterations) and the PE matmul stream
(~164us of column-cycles) are the two near-equal rooflines, so all QKV /
projection matmuls are streamed INTO the attention loop's PE slack instead
of running in separate phases where the other engine would idle. Blocks run
pr-major ((qc,pr0) x4 then (qc,pr1) x4) so only block 0 carries forced k/v
emissions. Input DMA is a few large transfers on both HWDGE rings (SP+ACT)
ordered so the first score matmul can start ~3us in.

Per-core layouts (contraction dim on SBUF partitions; host pre-transposes x):
  xt   [1024, 2048]  x[b].T
  wqk  [1024, 512]   w_qkv columns for this core's q (256) ++ k (256)
  wv   [1024, 256]   w_qkv columns for this core's v
  wpb  [256, 1024]   w_proj rows for this core's heads (perm'd, see host code)
  bc   [128, 2]      bc[p, m] = b_proj[g*256 + m*128 + p]
  out  [256, 2048]   rows g*256:(g+1)*256 of (x[b] @ ... ).T
"""
import sys

if '/opt/trn_rl_repo' not in sys.path:
    sys.path.insert(0, '/opt/trn_rl_repo')

import numpy as np

import concourse.bass as bass
import concourse.mybir as mybir
import concourse.tile as tile
from concourse import bacc
from concourse.bass_utils import run_bass_kernel_spmd

F32 = mybir.dt.float32
F16 = mybir.dt.float16

B = 2
N = 2048          # sequence length
C = 1024          # model dim
HD = 64           # head dim
SCALE = HD ** -0.5
NT = N // 128     # 16 key tiles
CT = C // 128     # 8 contraction tiles
QC = 4            # q-chunks of 512
QCS = N // QC     # 512
GROUPS = [[0, 1, 2, 3], [4, 5, 6, 7]]
NITER = 8 * NT    # 8 blocks x 16 key tiles

_NC_CACHE = None


def _blk(b):
    """qc-major block order: b = 2*qc + pr."""
    return (b // 2, b % 2)  # (qc, pr)


def build():
    nc = bacc.Bacc(None, target_bir_lowering=False, debug=False)

    # p-major host layouts: every input DMA moves ~128 multi-KB descriptors
    # (HWDGE issue time scales with descriptor count)
    xt_ext = nc.declare_dram_parameter("xt", [QC, 128, CT, QCS], F16, isOutput=False)
    wqk_ext = nc.declare_dram_parameter("wqk", [4, 128, CT, 128], F16, isOutput=False)
    wv_ext = nc.declare_dram_parameter("wv", [128, CT, 256], F16, isOutput=False)
    wpc_ext = nc.declare_dram_parameter("wpc", [128, CT, 256], F16, isOutput=False)
    bc_ext = nc.declare_dram_parameter("bc", [128, 2], F32, isOutput=False)
    out_ext = nc.declare_dram_parameter("out", [256, N], F16, isOutput=True)

    with tile.TileContext(nc) as tc:
        with (
            tc.tile_pool(name="weights", bufs=1) as wpool,
            tc.tile_pool(name="acts", bufs=1) as apool,
            tc.tile_pool(name="expt", bufs=3) as epool,
            tc.tile_pool(name="norm", bufs=2) as npool,
            tc.tile_pool(name="outp", bufs=2) as opool,
            tc.tile_pool(name="ofp", bufs=2) as ofpool,
            tc.tile_pool(name="psS", bufs=2, space="PSUM") as psS_pool,
            tc.tile_pool(name="psE", bufs=2, space="PSUM") as psE_pool,
            tc.tile_pool(name="psO", bufs=1, space="PSUM") as psO_pool,
            tc.tile_pool(name="dramog", bufs=2, space="DRAM") as og_pool,
            tc.tile_pool(name="dramag", bufs=8, space="DRAM") as ag_pool,
        ):
            # ---- SBUF tiles ----
            # xt/wqk chunk-major so each input DMA lands in a contiguous
            # per-partition region (large descriptors)
            xt_sb = apool.tile([128, QC, CT, QCS], F16, tag="xt")
            wqk_sb = wpool.tile([128, 4, CT, 128], F16, tag="wqk")
            wv_sb = wpool.tile([128, CT, 256], F16, tag="wv")
            wp_sb = wpool.tile([128, CT, 256], F16, tag="wp")
            bc_sb = wpool.tile([128, 2], F32, tag="bc")
            ones_row = wpool.tile([1, 64], F16, tag="ones_row")
            qk_sb = apool.tile([128, 4, N], F16, tag="qk")
            v_sb = apool.tile([128, NT, 4, 128], F16, tag="v")

            # constant fills on DVE (a DMA would be thousands of descriptors);
            # v columns 65:128 are never read (PV stationary is 65 cols wide)
            nc.vector.memset(ones_row[:, :], 1.0)
            nc.vector.memset(v_sb[:, :, :, HD:HD + 1], 1.0)

            # preload the Exp activation table while input DMAs stream
            dmy = npool.tile([1, 16], F16, tag="dmy")
            nc.vector.memset(dmy[:, :], 0.0)
            dmy2 = npool.tile([1, 16], F16, tag="dmy2")
            nc.scalar.activation(dmy2[:, :], dmy[:, :],
                                 mybir.ActivationFunctionType.Exp,
                                 bias=0.0, scale=1.0)

            # CC warmup: the first collective pays ~50us of one-time init on
            # the collective cores; run a dummy AllGather now (nothing reads
            # its result) so AG(0,0) executes promptly when issued.
            og_warm = og_pool.tile([1, 16], F16, tag="ogw", name="og_warm")
            nc.sync.dma_start(out=og_warm[:, :], in_=dmy[:, :])
            ag_warm = ag_pool.tile([4, 16], F16, tag="agw", name="ag_warm")
            nc.gpsimd.collective_compute(
                "AllGather", mybir.AluOpType.bypass, replica_groups=GROUPS,
                ins=[og_warm.opt()], outs=[ag_warm.opt()],
            )

            # ---- input DMAs: one per chunk, critical-path first, split over
            # both HWDGE rings (the wqk blocks finish on the ACT ring before
            # the first real exp needs the ACT sequencer).
            # wqk blocks: 0 = q pr0, 1 = q pr1, 2 = k pr0, 3 = k pr1.
            nc.scalar.dma_start(out=wqk_sb[:, 2, :, :], in_=wqk_ext.ap()[2])
            nc.scalar.dma_start(out=wqk_sb[:, 0, :, :], in_=wqk_ext.ap()[0])
            nc.scalar.dma_start(out=wqk_sb[:, 1, :, :], in_=wqk_ext.ap()[1])
            nc.scalar.dma_start(out=wqk_sb[:, 3, :, :], in_=wqk_ext.ap()[3])
            nc.sync.dma_start(out=xt_sb[:, 0, :, :], in_=xt_ext.ap()[0])
            nc.sync.dma_start(out=wv_sb[:, :, :], in_=wv_ext.ap())
            nc.sync.dma_start(out=xt_sb[:, 1, :, :], in_=xt_ext.ap()[1])
            nc.sync.dma_start(out=xt_sb[:, 2, :, :], in_=xt_ext.ap()[2])
            nc.sync.dma_start(out=xt_sb[:, 3, :, :], in_=xt_ext.ap()[3])
            nc.sync.dma_start(out=wp_sb[:, :, :], in_=wpc_ext.ap())
            nc.sync.dma_start(out=bc_sb[:, :], in_=bc_ext[:, :])

            # ---- emission helpers (PE work streamed into the loop) ----
            def xtcol(ct, kt):
                off = (kt % 4) * 128
                return xt_sb[:, kt // 4, ct, off:off + 128]

            def emit_kquad(pr, nch):
                """k for 4 key tiles at once: one stationary per ct streams a
                full 512-col xt chunk, so LDWEIGHTS hides under the matmul."""
                ksl = slice(nch * QCS, (nch + 1) * QCS)
                psq = psE_pool.tile([128, QCS], F32, tag="psE",
                                    name=f"psk_{pr}_{nch}")
                for ct in range(CT):
                    nc.tensor.matmul(
                        psq[:, :],
                        wqk_sb[:, 2 + pr, ct, :],
                        xt_sb[:, nch, ct, :],
                        start=(ct == 0), stop=(ct == CT - 1),
                    )
                nc.vector.tensor_copy(qk_sb[:, 2 + pr, ksl], psq[:, :])

            def emit_khalf(pr, nch, half):
                """k for 2 key tiles (256 cols) — shortens the cold prologue."""
                lo = nch * QCS + half * 256
                psq = psE_pool.tile([128, 256], F32, tag="psE",
                                    name=f"pskh_{pr}_{nch}_{half}")
                for ct in range(CT):
                    nc.tensor.matmul(
                        psq[:, :],
                        wqk_sb[:, 2 + pr, ct, :],
                        xt_sb[:, nch, ct, half * 256:(half + 1) * 256],
                        start=(ct == 0), stop=(ct == CT - 1),
                    )
                nc.vector.tensor_copy(qk_sb[:, 2 + pr, lo:lo + 256], psq[:, :])

            def emit_q(pr, qc):
                qsl = slice(qc * QCS, (qc + 1) * QCS)
                psq = psE_pool.tile([128, QCS], F32, tag="psE", name=f"psq_{pr}_{qc}")
                for ct in range(CT):
                    nc.tensor.matmul(
                        psq[:, :],
                        wqk_sb[:, pr, ct, :],
                        xt_sb[:, qc, ct, :],
                        start=(ct == 0), stop=(ct == CT - 1),
                    )
                nc.vector.tensor_copy(qk_sb[:, pr, qsl], psq[:, :])

            def v_mms(kt):
                """The 8 accumulation matmuls for v(kt), to be interleaved
                between long attention matmuls (hides their LDWEIGHTS)."""
                psv = psE_pool.tile([128, 256], F32, tag="psE", name=f"psv_{kt}")

                def mm(ct, psv=psv, kt=kt):
                    nc.tensor.matmul(
                        psv[:, :],
                        xtcol(ct, kt),
                        wv_sb[:, ct, :],
                        start=(ct == 0), stop=(ct == CT - 1),
                    )

                def fin(psv=psv, kt=kt):
                    nc.vector.tensor_copy(
                        v_sb[:, kt, :, 0:HD],
                        psv[:, :].rearrange("p (h e) -> p h e", h=4),
                    )
                return [lambda ct=ct: mm(ct) for ct in range(CT)], fin

            def scores(b, kt):
                qc, pr = _blk(b)
                qsl = slice(qc * QCS, (qc + 1) * QCS)
                ksl = slice(kt * 128, (kt + 1) * 128)
                psS = psS_pool.tile([128, 2 * QCS], F32, tag="psS",
                                    name=f"psS_{b}_{kt}")
                nc.tensor.matmul(
                    psS[:, 0:QCS],
                    qk_sb[0:64, 2 + pr, ksl],
                    qk_sb[0:64, pr, qsl],
                    start=True, stop=True,
                )
                nc.tensor.matmul(
                    psS[:, QCS:2 * QCS],
                    qk_sb[64:128, 2 + pr, ksl],
                    qk_sb[64:128, pr, qsl],
                    start=True, stop=True,
                )
                return psS

            # ---- per-block normalize / gather / project ----
            norm_state = {}
            ags = {}

            def part1(b):
                """Drain psO: o and rowsums to SBUF (frees psO for next block)."""
                psO = norm_state.pop(('psO', b))
                o2 = npool.tile([128, QCS], F32, tag="o2", name=f"o2_{b}")
                rs_e = npool.tile([1, QCS], F32, tag="rs_e", name=f"rse_{b}")
                rs_o = npool.tile([1, QCS], F32, tag="rs_o", name=f"rso_{b}")
                # bank A (head e) first so next block's first PV can start early
                nc.vector.tensor_copy(o2[0:64, :], psO[0:64, 0:QCS])
                nc.vector.tensor_copy(rs_e[:, :], psO[64:65, 0:QCS])
                nc.vector.tensor_copy(o2[64:128, :], psO[0:64, QCS:2 * QCS])
                nc.vector.tensor_copy(rs_o[:, :], psO[64:65, QCS:2 * QCS])
                norm_state[('o2', b)] = o2
                norm_state[('rs', b)] = (rs_e, rs_o)

            def part2(b):
                """1/rowsum (fast approx), broadcast via PE matmul, normalize."""
                o2 = norm_state.pop(('o2', b))
                rs_e, rs_o = norm_state.pop(('rs', b))
                # psB borrows a psS slot (psE may be held by split proj tiles)
                psB = psS_pool.tile([128, QCS], F32, tag="psS", name=f"psB_{b}")
                for hh, rs in ((0, rs_e), (1, rs_o)):
                    rcf = npool.tile([1, QCS], F32, tag="rcf", name=f"rcf_{b}_{hh}")
                    nc.vector.reciprocal_approx_fast(out=rcf[:, :], in_=rs[:, :])
                    rc16 = npool.tile([1, QCS], F16, tag="rc16", name=f"rc16_{b}_{hh}")
                    nc.vector.tensor_copy(rc16[:, :], rcf[:, :])
                    nc.tensor.matmul(psB[hh * 64:(hh + 1) * 64, :],
                                     ones_row[:, :], rc16[:, :],
                                     start=True, stop=True)
                on_sb = npool.tile([128, QCS], F16, tag="on", name=f"on_{b}")
                nc.vector.tensor_mul(on_sb[:, :], o2[:, :], psB[:, :])
                norm_state[('on', b)] = on_sb

            def part3(b):
                """Store + AllGather this block's head outputs."""
                qc, pr = _blk(b)
                on_sb = norm_state.pop(('on', b))
                og = og_pool.tile([128, QCS], F16, tag="og", name=f"og_{b}")
                nc.sync.dma_start(out=og[:, :], in_=on_sb[:, :])
                ag = ag_pool.tile([512, QCS], F16, tag="ag", name=f"ag_{b}")
                nc.gpsimd.collective_compute(
                    "AllGather",
                    mybir.AluOpType.bypass,
                    replica_groups=GROUPS,
                    ins=[og.opt()],
                    outs=[ag.opt()],
                )
                ags[(qc, pr)] = ag

            of_sbs = {}
            proj_ps = {}

            def emit_ofload(qc, half=None):
                if qc in of_sbs:
                    of_sb = of_sbs[qc]
                else:
                    of_sb = ofpool.tile([128, CT, QCS], F16, tag="of",
                                        name=f"of_{qc}")
                    of_sbs[qc] = of_sb
                prs = range(2) if half is None else [half]
                for pr in prs:
                    ag_r = ags[(qc, pr)][:, :].rearrange("(t p) n -> p t n", p=128)
                    nc.sync.dma_start(out=of_sb[:, pr * 4:(pr + 1) * 4, :], in_=ag_r)

            def emit_proj(qc, m2, lo=0, hi=CT):
                qsl = slice(qc * QCS, (qc + 1) * QCS)
                of_sb = of_sbs[qc]
                if (qc, m2) in proj_ps:
                    psP = proj_ps[(qc, m2)]
                else:
                    psP = psE_pool.tile([128, QCS], F32, tag="psE",
                                        name=f"psP_{qc}_{m2}")
                    proj_ps[(qc, m2)] = psP
                for t in range(lo, hi):
                    nc.tensor.matmul(
                        psP[:, :],
                        wp_sb[:, t, m2 * 128:(m2 + 1) * 128],
                        of_sb[:, t, :],
                        start=(t == 0), stop=(t == CT - 1),
                    )
                if hi == CT:
                    outsb = opool.tile([128, QCS], F16, tag="outsb",
                                       name=f"outsb_{qc}_{m2}")
                    nc.vector.tensor_scalar_add(outsb[:, :], psP[:, :],
                                                bc_sb[:, m2:m2 + 1])
                    nc.sync.dma_start(out=out_ext[m2 * 128:(m2 + 1) * 128, qsl],
                                      in_=outsb[:, :])

            # ---- static emission schedule: iter -> list of thunks ----
            sched = {}

            def at(i, fn):
                sched.setdefault(i, []).append(fn)

            at(0, lambda: emit_khalf(0, 0, 1))    # k(pr0) kt2-3, due iter 1
            at(1, lambda: emit_kquad(0, 1))       # k(pr0) due iters 4/8/12
            at(5, lambda: emit_kquad(0, 2))
            at(9, lambda: emit_kquad(0, 3))
            at(11, lambda: emit_q(1, 0))          # due block 1 (iter 15)
            at(12, lambda: emit_kquad(1, 0))      # k(pr1) due iter 16/20/24/28
            at(17, lambda: emit_kquad(1, 1))
            at(21, lambda: emit_kquad(1, 2))
            at(25, lambda: emit_kquad(1, 3))
            at(27, lambda: emit_q(0, 1))          # due block 2 (iter 31)
            at(43, lambda: emit_q(1, 1))          # due block 3 (iter 47)
            at(59, lambda: emit_q(0, 2))          # due block 4 (iter 63)
            at(75, lambda: emit_q(1, 2))          # due block 5 (iter 79)
            at(91, lambda: emit_q(0, 3))          # due block 6 (iter 95)
            at(107, lambda: emit_q(1, 3))         # due block 7 (iter 111)
            for qc in range(3):                   # proj(qc): AG(qc,1) done at
                at(32 * qc + 49, lambda qc=qc: emit_ofload(qc))   # ~iter 38+32qc
                at(32 * qc + 52, lambda qc=qc: emit_proj(qc, 0))
                at(32 * qc + 54, lambda qc=qc: emit_proj(qc, 1))
            # proj(3): pr0-half of the contraction runs before the last AG
            at(120, lambda: emit_ofload(3, half=0))
            at(122, lambda: emit_proj(3, 0, 0, 4))
            at(124, lambda: emit_proj(3, 1, 0, 4))
            for b in range(7):                    # normalize tail of each block
                at(16 * b + 17, lambda b=b: part2(b))
                at(16 * b + 18, lambda b=b: part3(b))

            # v(kt) streamed during block 0, matmuls interleaved between the
            # long attention matmuls so each LDWEIGHTS hides under them
            v_plan = {kt - 1: kt for kt in range(1, NT)}

            # ---- prologue (kept short: it runs at the cold PE clock; v(0)
            # comes after the first scores so exp(0) starts sooner) ----
            emit_khalf(0, 0, 0)
            emit_q(0, 0)
            psS_cur = scores(0, 0)
            vm, vfin = v_mms(0)
            for m in vm:
                m()
            vfin()

            # ---- main loop ----
            for bi in range(NITER):
                b, kt = bi // NT, bi % NT
                qc, pr = _blk(b)
                vkt = v_plan.get(bi)
                vm, vfin = v_mms(vkt) if vkt is not None else ([], None)
                if bi + 1 < NITER:
                    nb, nkt = (b, kt + 1) if kt < NT - 1 else (b + 1, 0)
                    nqc, npr = _blk(nb)
                    qsl_ = slice(nqc * QCS, (nqc + 1) * QCS)
                    ksl_ = slice(nkt * 128, (nkt + 1) * 128)
                    psS_next = psS_pool.tile([128, 2 * QCS], F32, tag="psS",
                                             name=f"psS_{nb}_{nkt}")
                    nc.tensor.matmul(psS_next[:, 0:QCS],
                                     qk_sb[0:64, 2 + npr, ksl_],
                                     qk_sb[0:64, npr, qsl_],
                                     start=True, stop=True)
                    for m in vm[0:2]:
                        m()
                    nc.tensor.matmul(psS_next[:, QCS:2 * QCS],
                                     qk_sb[64:128, 2 + npr, ksl_],
                                     qk_sb[64:128, npr, qsl_],
                                     start=True, stop=True)
                    for m in vm[2:4]:
                        m()
                else:
                    psS_next = None
                    for m in vm[0:4]:
                        m()
                expt = epool.tile([128, 2 * QCS], F16, tag="expt",
                                  name=f"expt_{bi}")
                nc.scalar.activation(
                    expt[:, :], psS_cur[:, :],
                    mybir.ActivationFunctionType.Exp,
                    bias=0.0, scale=SCALE,
                )
                for fn in sched.get(bi, ()):
                    fn()
                if kt == 0:
                    norm_state[('psO', b)] = psO_pool.tile(
                        [128, 2 * QCS], F32, tag="psO", name=f"psO_{b}")
                psO = norm_state[('psO', b)]
                nc.tensor.matmul(
                    psO[0:65, 0:QCS],
                    v_sb[:, kt, 2 * pr, 0:65],
                    expt[:, 0:QCS],
                    start=(kt == 0), stop=(kt == NT - 1),
                )
                for m in vm[4:6]:
                    m()
                nc.tensor.matmul(
                    psO[0:65, QCS:2 * QCS],
                    v_sb[:, kt, 2 * pr + 1, 0:65],
                    expt[:, QCS:2 * QCS],
                    start=(kt == 0), stop=(kt == NT - 1),
                )
                for m in vm[6:8]:
                    m()
                if vfin is not None:
                    vfin()
                psS_cur = psS_next
                if kt == NT - 1:
                    part1(b)

            # ---- tail: last block's normalize + gather + projection ----
            part2(7)
            part3(7)
            emit_ofload(3, half=1)
            emit_proj(3, 0, 4, CT)
            emit_proj(3, 1, 4, CT)

    nc.compile()
    return nc


def _get_nc():
    global _NC_CACHE
    if _NC_CACHE is None:
        _NC_CACHE = build()
    return _NC_CACHE


def shard_inputs(x, w_qkv, w_proj, b_proj):
    x = np.asarray(x, dtype=np.float32)
    w_qkv = np.asarray(w_qkv, dtype=np.float32)
    w_proj = np.asarray(w_proj, dtype=np.float32)
    b_proj = np.asarray(b_proj, dtype=np.float32)
    # ag row order: for each pr, rank-major then local-head-major:
    # rows [j*128 + h2*64 + e] <-> global head 4j + 2*pr + h2
    perm = np.concatenate([
        np.arange(1024).reshape(16, 64)[[4 * j + 2 * pr + h2 for j in range(4) for h2 in range(2)]].reshape(-1)
        for pr in range(2)
    ])
    def pmajor_kt(w):
        # [C, M] -> [128(p), CT(t), M]: row t*128+p -> [p, t]
        return np.ascontiguousarray(
            w.reshape(CT, 128, w.shape[1]).transpose(1, 0, 2).astype(np.float16))

    in_maps = []
    for core in range(8):
        b, g = divmod(core, 4)
        cs = slice(g * 256, (g + 1) * 256)
        xtT = x[b].T  # [C, N]
        # xt: [QC(nch), 128(p), CT(t), QCS] with [nch,p,t,c] = xtT[t*128+p, nch*512+c]
        xt_arr = np.ascontiguousarray(
            xtT.reshape(CT, 128, QC, QCS).transpose(2, 1, 0, 3).astype(np.float16))
        # wqk blocks: 0 = q pr0, 1 = q pr1, 2 = k pr0, 3 = k pr1 (128 cols each)
        qcols = w_qkv[:, 0 * C + g * 256:0 * C + (g + 1) * 256]
        kcols = w_qkv[:, 1 * C + g * 256:1 * C + (g + 1) * 256]
        wqk = np.concatenate([qcols, kcols], axis=1)  # [C, 512]
        wqk_arr = np.ascontiguousarray(
            wqk.reshape(CT, 128, 4, 128).transpose(2, 1, 0, 3).astype(np.float16))
        in_maps.append({
            "xt": xt_arr,
            "wqk": wqk_arr,
            "wv": pmajor_kt(w_qkv[:, 2 * C + g * 256:2 * C + (g + 1) * 256]),
            "wpc": pmajor_kt(w_proj[perm, :][:, cs]),
            "bc": np.ascontiguousarray(b_proj[cs].reshape(2, 128).T),
        })
    return in_maps


def assemble_output(results):
    outT = np.empty((B, C, N), dtype=np.float32)
    for core in range(8):
        b, g = divmod(core, 4)
        outT[b, g * 256:(g + 1) * 256, :] = np.asarray(results[core]["out"], dtype=np.float32)
    return np.ascontiguousarray(outT.transpose(0, 2, 1))


def run_sharded(x, w_qkv, w_proj, b_proj, trace=False):
    nc = _get_nc()
    in_maps = shard_inputs(x, w_qkv, w_proj, b_proj)
    res = run_bass_kernel_spmd(nc, in_maps, core_ids=list(range(8)), trace=trace)
    return assemble_output(res.results), res.exec_time_ns


def kernel(x, w_qkv, w_proj, b_proj):
    out, _ = run_sharded(x, w_qkv, w_proj, b_proj, trace=False)
    return out


# revision 33
# speedup vs baseline: 1.1767x; 1.0190x over previous
"""Multi-head attention block (B=2, N=2048, C=1024, H=16, hd=64) on 8 TRN2 NeuronCores.

Sharding: data-parallel over batch (2 groups of 4 cores), tensor-parallel over
heads within each group (4 heads/core). Each core computes q/k/v for its heads,
attention, and a partial output projection; an AllGather over the 4-core group
collects head outputs, and each core projects its 256-column slice.

v2 schedule: single fused loop. The softmax exp on the Scalar engine
(~1.3us per 1024 columns x 128 iterations) and the PE matmul stream
(~164us of column-cycles) are the two near-equal rooflines, so all QKV /
projection matmuls are streamed INTO the attention loop's PE slack instead
of running in separate phases where the other engine would idle. Blocks run
pr-major ((qc,pr0) x4 then (qc,pr1) x4) so only block 0 carries forced k/v
emissions. Input DMA is a few large transfers on both HWDGE rings (SP+ACT)
ordered so the first score matmul can start ~3us in.

Per-core layouts (contraction dim on SBUF partitions; host pre-transposes x):
  xt   [1024, 2048]  x[b].T
  wqk  [1024, 512]   w_qkv columns for this core's q (256) ++ k (256)
  wv   [1024, 256]   w_qkv columns for this core's v
  wpb  [256, 1024]   w_proj rows for this core's heads (perm'd, see host code)
  bc   [128, 2]      bc[p, m] = b_proj[g*256 + m*128 + p]
  out  [256, 2048]   rows g*256:(g+1)*256 of (x[b] @ ... ).T
"""
import sys

if '/opt/trn_rl_repo' not in sys.path:
    sys.path.insert(0, '/opt/trn_rl_repo')

import numpy as np

import concourse.bass as bass
import concourse.mybir as mybir
import concourse.tile as tile
from concourse import bacc
from concourse.bass_utils import run_bass_kernel_spmd

F32 = mybir.dt.float32
F16 = mybir.dt.float16

B = 2
N = 2048          # sequence length
C = 1024          # model dim
HD = 64           # head dim
SCALE = HD ** -0.5
NT = N // 128     # 16 key tiles
CT = C // 128     # 8 contraction tiles
QC = 4            # q-chunks of 512
QCS = N // QC     # 512
GROUPS = [[0, 1, 2, 3], [4, 5, 6, 7]]
NITER = 8 * NT    # 8 blocks x 16 key tiles

_NC_CACHE = None


def _blk(b):
    """qc-major block order: b = 2*qc + pr."""
    return (b // 2, b % 2)  # (qc, pr)


def build():
    nc = bacc.Bacc(None, target_bir_lowering=False, debug=False)

    # p-major host layouts: every input DMA moves ~128 multi-KB descriptors
    # (HWDGE issue time scales with descriptor count)
    xt_ext = nc.declare_dram_parameter("xt", [QC, 128, CT, QCS], F16, isOutput=False)
    wqk_ext = nc.declare_dram_parameter("wqk", [4, 128, CT, 128], F16, isOutput=False)
    wv_ext = nc.declare_dram_parameter("wv", [128, CT, 256], F16, isOutput=False)
    wpc_ext = nc.declare_dram_parameter("wpc", [128, CT, 256], F16, isOutput=False)
    bc_ext = nc.declare_dram_parameter("bc", [128, 2], F32, isOutput=False)
    out_ext = nc.declare_dram_parameter("out", [256, N], F16, isOutput=True)

    with tile.TileContext(nc) as tc:
        with (
            tc.tile_pool(name="weights", bufs=1) as wpool,
            tc.tile_pool(name="acts", bufs=1) as apool,
            tc.tile_pool(name="expt", bufs=3) as epool,
            tc.tile_pool(name="norm", bufs=2) as npool,
            tc.tile_pool(name="outp", bufs=2) as opool,
            tc.tile_pool(name="ofp", bufs=2) as ofpool,
            tc.tile_pool(name="psS", bufs=2, space="PSUM") as psS_pool,
            tc.tile_pool(name="psE", bufs=2, space="PSUM") as psE_pool,
            tc.tile_pool(name="psO", bufs=1, space="PSUM") as psO_pool,
            tc.tile_pool(name="dramog", bufs=2, space="DRAM") as og_pool,
            tc.tile_pool(name="dramag", bufs=8, space="DRAM") as ag_pool,
        ):
            # ---- SBUF tiles ----
            # xt/wqk chunk-major so each input DMA lands in a contiguous
            # per-partition region (large descriptors)
            xt_sb = apool.tile([128, QC, CT, QCS], F16, tag="xt")
            wqk_sb = wpool.tile([128, 4, CT, 128], F16, tag="wqk")
            wv_sb = wpool.tile([128, CT, 256], F16, tag="wv")
            wp_sb = wpool.tile([128, CT, 256], F16, tag="wp")
            bc_sb = wpool.tile([128, 2], F32, tag="bc")
            ones_row = wpool.tile([1, 64], F16, tag="ones_row")
            qk_sb = apool.tile([128, 4, N], F16, tag="qk")
            v_sb = apool.tile([128, NT, 4, 128], F16, tag="v")

            # constant fills on DVE (a DMA would be thousands of descriptors);
            # v columns 65:128 are never read (PV stationary is 65 cols wide)
            nc.vector.memset(ones_row[:, :], 1.0)
            nc.vector.memset(v_sb[:, :, :, HD:HD + 1], 1.0)

            # preload the Exp activation table while input DMAs stream
            dmy = npool.tile([1, 16], F16, tag="dmy")
            nc.vector.memset(dmy[:, :], 0.0)
            dmy2 = npool.tile([1, 16], F16, tag="dmy2")
            nc.scalar.activation(dmy2[:, :], dmy[:, :],
                                 mybir.ActivationFunctionType.Exp,
                                 bias=0.0, scale=1.0)

            # CC warmup: the first collective pays ~50us of one-time init on
            # the collective cores; run a dummy AllGather now (nothing reads
            # its result) so AG(0,0) executes promptly when issued.
            og_warm = og_pool.tile([1, 16], F16, tag="ogw", name="og_warm")
            nc.sync.dma_start(out=og_warm[:, :], in_=dmy[:, :])
            ag_warm = ag_pool.tile([4, 16], F16, tag="agw", name="ag_warm")
            nc.gpsimd.collective_compute(
                "AllGather", mybir.AluOpType.bypass, replica_groups=GROUPS,
                ins=[og_warm.opt()], outs=[ag_warm.opt()],
            )

            # ---- input DMAs: one per chunk, critical-path first, split over
            # both HWDGE rings (the wqk blocks finish on the ACT ring before
            # the first real exp needs the ACT sequencer).
            # wqk blocks: 0 = q pr0, 1 = q pr1, 2 = k pr0, 3 = k pr1.
            nc.scalar.dma_start(out=wqk_sb[:, 2, :, :], in_=wqk_ext.ap()[2])
            nc.scalar.dma_start(out=wqk_sb[:, 0, :, :], in_=wqk_ext.ap()[0])
            nc.scalar.dma_start(out=wqk_sb[:, 1, :, :], in_=wqk_ext.ap()[1])
            nc.scalar.dma_start(out=wqk_sb[:, 3, :, :], in_=wqk_ext.ap()[3])
            nc.sync.dma_start(out=xt_sb[:, 0, :, :], in_=xt_ext.ap()[0])
            nc.sync.dma_start(out=wv_sb[:, :, :], in_=wv_ext.ap())
            nc.sync.dma_start(out=xt_sb[:, 1, :, :], in_=xt_ext.ap()[1])
            nc.sync.dma_start(out=xt_sb[:, 2, :, :], in_=xt_ext.ap()[2])
            nc.sync.dma_start(out=xt_sb[:, 3, :, :], in_=xt_ext.ap()[3])
            nc.sync.dma_start(out=wp_sb[:, :, :], in_=wpc_ext.ap())
            nc.sync.dma_start(out=bc_sb[:, :], in_=bc_ext[:, :])

            # ---- emission helpers (PE work streamed into the loop) ----
            def xtcol(ct, kt):
                off = (kt % 4) * 128
                return xt_sb[:, kt // 4, ct, off:off + 128]

            def emit_kquad(pr, nch):
                """k for 4 key tiles at once: one stationary per ct streams a
                full 512-col xt chunk, so LDWEIGHTS hides under the matmul."""
                ksl = slice(nch * QCS, (nch + 1) * QCS)
                psq = psE_pool.tile([128, QCS], F32, tag="psE",
                                    name=f"psk_{pr}_{nch}")
                for ct in range(CT):
                    nc.tensor.matmul(
                        psq[:, :],
                        wqk_sb[:, 2 + pr, ct, :],
                        xt_sb[:, nch, ct, :],
                        start=(ct == 0), stop=(ct == CT - 1),
                    )
                nc.vector.tensor_copy(qk_sb[:, 2 + pr, ksl], psq[:, :])

            def emit_khalf(pr, nch, half):
                """k for 2 key tiles (256 cols) — shortens the cold prologue."""
                lo = nch * QCS + half * 256
                psq = psE_pool.tile([128, 256], F32, tag="psE",
                                    name=f"pskh_{pr}_{nch}_{half}")
                for ct in range(CT):
                    nc.tensor.matmul(
                        psq[:, :],
                        wqk_sb[:, 2 + pr, ct, :],
                        xt_sb[:, nch, ct, half * 256:(half + 1) * 256],
                        start=(ct == 0), stop=(ct == CT - 1),
                    )
                nc.vector.tensor_copy(qk_sb[:, 2 + pr, lo:lo + 256], psq[:, :])

            def emit_q(pr, qc):
                qsl = slice(qc * QCS, (qc + 1) * QCS)
                psq = psE_pool.tile([128, QCS], F32, tag="psE", name=f"psq_{pr}_{qc}")
                for ct in range(CT):
                    nc.tensor.matmul(
                        psq[:, :],
                        wqk_sb[:, pr, ct, :],
                        xt_sb[:, qc, ct, :],
                        start=(ct == 0), stop=(ct == CT - 1),
                    )
                nc.vector.tensor_copy(qk_sb[:, pr, qsl], psq[:, :])

            def v_mms(kt):
                """The 8 accumulation matmuls for v(kt), to be interleaved
                between long attention matmuls (hides their LDWEIGHTS)."""
                psv = psE_pool.tile([128, 256], F32, tag="psE", name=f"psv_{kt}")

                def mm(ct, psv=psv, kt=kt):
                    nc.tensor.matmul(
                        psv[:, :],
                        xtcol(ct, kt),
                        wv_sb[:, ct, :],
                        start=(ct == 0), stop=(ct == CT - 1),
                    )

                def fin(psv=psv, kt=kt):
                    nc.vector.tensor_copy(
                        v_sb[:, kt, :, 0:HD],
                        psv[:, :].rearrange("p (h e) -> p h e", h=4),
                    )
                return [lambda ct=ct: mm(ct) for ct in range(CT)], fin

            def scores(b, kt):
                qc, pr = _blk(b)
                qsl = slice(qc * QCS, (qc + 1) * QCS)
                ksl = slice(kt * 128, (kt + 1) * 128)
                psS = psS_pool.tile([128, 2 * QCS], F32, tag="psS",
                                    name=f"psS_{b}_{kt}")
                nc.tensor.matmul(
                    psS[:, 0:QCS],
                    qk_sb[0:64, 2 + pr, ksl],
                    qk_sb[0:64, pr, qsl],
                    start=True, stop=True,
                )
                nc.tensor.matmul(
                    psS[:, QCS:2 * QCS],
                    qk_sb[64:128, 2 + pr, ksl],
                    qk_sb[64:128, pr, qsl],
                    start=True, stop=True,
                )
                return psS

            # ---- per-block normalize / gather / project ----
            norm_state = {}
            ags = {}

            def part1(b):
                """Drain psO (o + rowsum rows) to SBUF; frees psO for the
                next block after just two DVE copies."""
                psO = norm_state.pop(('psO', b))
                o2x = npool.tile([65, QCS], F32, tag="o2x", name=f"o2x_{b}")
                o2y = npool.tile([65, QCS], F32, tag="o2y", name=f"o2y_{b}")
                nc.vector.tensor_copy(o2x[:, :], psO[0:65, 0:QCS])
                nc.vector.tensor_copy(o2y[:, :], psO[0:65, QCS:2 * QCS])
                norm_state[('o2', b)] = (o2x, o2y)

            def part2(b):
                """1/rowsum (fast approx), broadcast via PE matmul, normalize."""
                o2x, o2y = norm_state.pop(('o2', b))
                # block 7's psB must not take a psE slot (the split proj(3)
                # tiles hold both psE slots through the tail -> deadlock);
                # psS is free by then instead
                if b == 7:
                    psB = psS_pool.tile([128, QCS], F32, tag="psS",
                                        name=f"psB_{b}")
                else:
                    psB = psE_pool.tile([128, QCS], F32, tag="psE",
                                        name=f"psB_{b}")
                for hh, o2 in ((0, o2x), (1, o2y)):
                    rs = npool.tile([1, QCS], F32, tag="rs", name=f"rs_{b}_{hh}")
                    nc.vector.tensor_copy(rs[:, :], o2[64:65, :])
                    rcf = npool.tile([1, QCS], F32, tag="rcf", name=f"rcf_{b}_{hh}")
                    nc.vector.reciprocal_approx_fast(out=rcf[:, :], in_=rs[:, :])
                    rc16 = npool.tile([1, QCS], F16, tag="rc16", name=f"rc16_{b}_{hh}")
                    nc.vector.tensor_copy(rc16[:, :], rcf[:, :])
                    nc.tensor.matmul(psB[hh * 64:(hh + 1) * 64, :],
                                     ones_row[:, :], rc16[:, :],
                                     start=True, stop=True)
                on_sb = npool.tile([128, QCS], F16, tag="on", name=f"on_{b}")
                nc.vector.tensor_mul(on_sb[0:64, :], o2x[0:64, :], psB[0:64, :])
                nc.vector.tensor_mul(on_sb[64:128, :], o2y[0:64, :],
                                     psB[64:128, :])
                norm_state[('on', b)] = on_sb

            def part3(b):
                """Store + AllGather this block's head outputs."""
                qc, pr = _blk(b)
                on_sb = norm_state.pop(('on', b))
                og = og_pool.tile([128, QCS], F16, tag="og", name=f"og_{b}")
                nc.sync.dma_start(out=og[:, :], in_=on_sb[:, :])
                ag = ag_pool.tile([512, QCS], F16, tag="ag", name=f"ag_{b}")
                nc.gpsimd.collective_compute(
                    "AllGather",
                    mybir.AluOpType.bypass,
                    replica_groups=GROUPS,
                    ins=[og.opt()],
                    outs=[ag.opt()],
                )
                ags[(qc, pr)] = ag

            of_sbs = {}
            proj_ps = {}

            def emit_ofload(qc, half=None):
                if qc in of_sbs:
                    of_sb = of_sbs[qc]
                else:
                    of_sb = ofpool.tile([128, CT, QCS], F16, tag="of",
                                        name=f"of_{qc}")
                    of_sbs[qc] = of_sb
                prs = range(2) if half is None else [half]
                for pr in prs:
                    ag_r = ags[(qc, pr)][:, :].rearrange("(t p) n -> p t n", p=128)
                    nc.sync.dma_start(out=of_sb[:, pr * 4:(pr + 1) * 4, :], in_=ag_r)

            def emit_proj(qc, m2, lo=0, hi=CT):
                qsl = slice(qc * QCS, (qc + 1) * QCS)
                of_sb = of_sbs[qc]
                if (qc, m2) in proj_ps:
                    psP = proj_ps[(qc, m2)]
                else:
                    psP = psE_pool.tile([128, QCS], F32, tag="psE",
                                        name=f"psP_{qc}_{m2}")
                    proj_ps[(qc, m2)] = psP
                for t in range(lo, hi):
                    nc.tensor.matmul(
                        psP[:, :],
                        wp_sb[:, t, m2 * 128:(m2 + 1) * 128],
                        of_sb[:, t, :],
                        start=(t == 0), stop=(t == CT - 1),
                    )
                if hi == CT:
                    outsb = opool.tile([128, QCS], F16, tag="outsb",
                                       name=f"outsb_{qc}_{m2}")
                    nc.vector.tensor_scalar_add(outsb[:, :], psP[:, :],
                                                bc_sb[:, m2:m2 + 1])
                    nc.sync.dma_start(out=out_ext[m2 * 128:(m2 + 1) * 128, qsl],
                                      in_=outsb[:, :])

            # ---- static emission schedule: iter -> list of thunks ----
            sched = {}

            def at(i, fn):
                sched.setdefault(i, []).append(fn)

            at(0, lambda: emit_khalf(0, 0, 1))    # k(pr0) kt2-3, due iter 1
            at(1, lambda: emit_kquad(0, 1))       # k(pr0) due iters 4/8/12
            at(5, lambda: emit_kquad(0, 2))
            at(9, lambda: emit_kquad(0, 3))
            at(11, lambda: emit_q(1, 0))          # due block 1 (iter 15)
            at(12, lambda: emit_kquad(1, 0))      # k(pr1) due iter 16/20/24/28
            at(17, lambda: emit_kquad(1, 1))
            at(21, lambda: emit_kquad(1, 2))
            at(25, lambda: emit_kquad(1, 3))
            at(27, lambda: emit_q(0, 1))          # due block 2 (iter 31)
            at(43, lambda: emit_q(1, 1))          # due block 3 (iter 47)
            at(59, lambda: emit_q(0, 2))          # due block 4 (iter 63)
            at(75, lambda: emit_q(1, 2))          # due block 5 (iter 79)
            at(91, lambda: emit_q(0, 3))          # due block 6 (iter 95)
            at(107, lambda: emit_q(1, 3))         # due block 7 (iter 111)
            for qc in range(3):                   # proj(qc): AG(qc,1) done at
                at(32 * qc + 49, lambda qc=qc: emit_ofload(qc))   # ~iter 38+32qc
                at(32 * qc + 52, lambda qc=qc: emit_proj(qc, 0))
                at(32 * qc + 54, lambda qc=qc: emit_proj(qc, 1))
            # proj(3): pr0-half of the contraction runs before the last AG
            at(120, lambda: emit_ofload(3, half=0))
            at(122, lambda: emit_proj(3, 0, 0, 4))
            at(124, lambda: emit_proj(3, 1, 0, 4))
            for b in range(7):                    # normalize tail of each block
                at(16 * b + 17, lambda b=b: part2(b))
                at(16 * b + 18, lambda b=b: part3(b))

            # v(kt) streamed during block 0, matmuls interleaved between the
            # long attention matmuls so each LDWEIGHTS hides under them
            v_plan = {kt - 1: kt for kt in range(1, NT)}

            # ---- prologue (kept short: it runs at the cold PE clock; v(0)
            # comes after the first scores so exp(0) starts sooner) ----
            emit_khalf(0, 0, 0)
            emit_q(0, 0)
            psS_cur = scores(0, 0)
            vm, vfin = v_mms(0)
            for m in vm:
                m()
            vfin()

            # ---- main loop ----
            for bi in range(NITER):
                b, kt = bi // NT, bi % NT
                qc, pr = _blk(b)
                vkt = v_plan.get(bi)
                vm, vfin = v_mms(vkt) if vkt is not None else ([], None)
                if bi + 1 < NITER:
                    nb, nkt = (b, kt + 1) if kt < NT - 1 else (b + 1, 0)
                    nqc, npr = _blk(nb)
                    qsl_ = slice(nqc * QCS, (nqc + 1) * QCS)
                    ksl_ = slice(nkt * 128, (nkt + 1) * 128)
                    psS_next = psS_pool.tile([128, 2 * QCS], F32, tag="psS",
                                             name=f"psS_{nb}_{nkt}")
                    nc.tensor.matmul(psS_next[:, 0:QCS],
                                     qk_sb[0:64, 2 + npr, ksl_],
                                     qk_sb[0:64, npr, qsl_],
                                     start=True, stop=True)
                    for m in vm[0:2]:
                        m()
                    nc.tensor.matmul(psS_next[:, QCS:2 * QCS],
                                     qk_sb[64:128, 2 + npr, ksl_],
                                     qk_sb[64:128, npr, qsl_],
                                     start=True, stop=True)
                    for m in vm[2:4]:
                        m()
                else:
                    psS_next = None
                    for m in vm[0:4]:
                        m()
                expt = epool.tile([128, 2 * QCS], F16, tag="expt",
                                  name=f"expt_{bi}")
                nc.scalar.activation(
                    expt[:, :], psS_cur[:, :],
                    mybir.ActivationFunctionType.Exp,
                    bias=0.0, scale=SCALE,
                )
                for fn in sched.get(bi, ()):
                    fn()
                if kt == 0:
                    norm_state[('psO', b)] = psO_pool.tile(
                        [128, 2 * QCS], F32, tag="psO", name=f"psO_{b}")
                psO = norm_state[('psO', b)]
                nc.tensor.matmul(
                    psO[0:65, 0:QCS],
                    v_sb[:, kt, 2 * pr, 0:65],
                    expt[:, 0:QCS],
                    start=(kt == 0), stop=(kt == NT - 1),
                )
                for m in vm[4:6]:
                    m()
                nc.tensor.matmul(
                    psO[0:65, QCS:2 * QCS],
                    v_sb[:, kt, 2 * pr + 1, 0:65],
                    expt[:, QCS:2 * QCS],
                    start=(kt == 0), stop=(kt == NT - 1),
                )
                for m in vm[6:8]:
                    m()
                if vfin is not None:
                    vfin()
                psS_cur = psS_next
                if kt == NT - 1:
                    part1(b)

            # ---- tail: last block's normalize + gather + projection ----
            part2(7)
            part3(7)
            # last of-load split across both HWDGE rings (ACT is idle now)
            ag_r3 = ags[(3, 1)][:, :].rearrange("(t p) n -> p t n", p=128)
            of3 = of_sbs[3]
            nc.sync.dma_start(out=of3[:, 4:6, :], in_=ag_r3[:, 0:2, :])
            nc.scalar.dma_start(out=of3[:, 6:8, :], in_=ag_r3[:, 2:4, :])
            emit_proj(3, 0, 4, CT)
            emit_proj(3, 1, 4, CT)

    nc.compile()
    return nc


def _get_nc():
    global _NC_CACHE
    if _NC_CACHE is None:
        _NC_CACHE = build()
    return _NC_CACHE


def shard_inputs(x, w_qkv, w_proj, b_proj):
    x = np.asarray(x, dtype=np.float32)
    w_qkv = np.asarray(w_qkv, dtype=np.float32)
    w_proj = np.asarray(w_proj, dtype=np.float32)
    b_proj = np.asarray(b_proj, dtype=np.float32)
    # ag row order: for each pr, rank-major then local-head-major:
    # rows [j*128 + h2*64 + e] <-> global head 4j + 2*pr + h2
    perm = np.concatenate([
        np.arange(1024).reshape(16, 64)[[4 * j + 2 * pr + h2 for j in range(4) for h2 in range(2)]].reshape(-1)
        for pr in range(2)
    ])
    def pmajor_kt(w):
        # [C, M] -> [128(p), CT(t), M]: row t*128+p -> [p, t]
        return np.ascontiguousarray(
            w.reshape(CT, 128, w.shape[1]).transpose(1, 0, 2).astype(np.float16))

    in_maps = []
    for core in range(8):
        b, g = divmod(core, 4)
        cs = slice(g * 256, (g + 1) * 256)
        xtT = x[b].T  # [C, N]
        # xt: [QC(nch), 128(p), CT(t), QCS] with [nch,p,t,c] = xtT[t*128+p, nch*512+c]
        xt_arr = np.ascontiguousarray(
            xtT.reshape(CT, 128, QC, QCS).transpose(2, 1, 0, 3).astype(np.float16))
        # wqk blocks: 0 = q pr0, 1 = q pr1, 2 = k pr0, 3 = k pr1 (128 cols each)
        qcols = w_qkv[:, 0 * C + g * 256:0 * C + (g + 1) * 256]
        kcols = w_qkv[:, 1 * C + g * 256:1 * C + (g + 1) * 256]
        wqk = np.concatenate([qcols, kcols], axis=1)  # [C, 512]
        wqk_arr = np.ascontiguousarray(
            wqk.reshape(CT, 128, 4, 128).transpose(2, 1, 0, 3).astype(np.float16))
        in_maps.append({
            "xt": xt_arr,
            "wqk": wqk_arr,
            "wv": pmajor_kt(w_qkv[:, 2 * C + g * 256:2 * C + (g + 1) * 256]),
            "wpc": pmajor_kt(w_proj[perm, :][:, cs]),
            "bc": np.ascontiguousarray(b_proj[cs].reshape(2, 128).T),
        })
    return in_maps


def assemble_output(results):
    outT = np.empty((B, C, N), dtype=np.float32)
    for core in range(8):
        b, g = divmod(core, 4)
        outT[b, g * 256:(g + 1) * 256, :] = np.asarray(results[core]["out"], dtype=np.float32)
    return np.ascontiguousarray(outT.transpose(0, 2, 1))


def run_sharded(x, w_qkv, w_proj, b_proj, trace=False):
    nc = _get_nc()
    in_maps = shard_inputs(x, w_qkv, w_proj, b_proj)
    res = run_bass_kernel_spmd(nc, in_maps, core_ids=list(range(8)), trace=trace)
    return assemble_output(res.results), res.exec_time_ns


def kernel(x, w_qkv, w_proj, b_proj):
    out, _ = run_sharded(x, w_qkv, w_proj, b_proj, trace=False)
    return out


# revision 39
# speedup vs baseline: 1.2804x; 1.0882x over previous
"""Multi-head attention block (B=2, N=2048, C=1024, H=16, hd=64) on 8 TRN2 NeuronCores.

Sharding: data-parallel over batch (2 groups of 4 cores), tensor-parallel over
heads within each group (4 heads/core). Each core computes q/k/v for its heads,
attention, and a partial output projection; an AllGather over the 4-core group
collects head outputs, and each core projects its 256-column slice.

v2 schedule: single fused loop. The softmax exp on the Scalar engine
(~1.3us per 1024 columns x 128 iterations) and the PE matmul stream
(~164us of column-cycles) are the two near-equal rooflines, so all QKV /
projection matmuls are streamed INTO the attention loop's PE slack instead
of running in separate phases where the other engine would idle. Blocks run
pr-major ((qc,pr0) x4 then (qc,pr1) x4) so only block 0 carries forced k/v
emissions. Input DMA is a few large transfers on both HWDGE rings (SP+ACT)
ordered so the first score matmul can start ~3us in.

Per-core layouts (contraction dim on SBUF partitions; host pre-transposes x):
  xt   [1024, 2048]  x[b].T
  wqk  [1024, 512]   w_qkv columns for this core's q (256) ++ k (256)
  wv   [1024, 256]   w_qkv columns for this core's v
  wpb  [256, 1024]   w_proj rows for this core's heads (perm'd, see host code)
  bc   [128, 2]      bc[p, m] = b_proj[g*256 + m*128 + p]
  out  [256, 2048]   rows g*256:(g+1)*256 of (x[b] @ ... ).T
"""
import sys

if '/opt/trn_rl_repo' not in sys.path:
    sys.path.insert(0, '/opt/trn_rl_repo')

import numpy as np

import concourse.bass as bass
import concourse.mybir as mybir
import concourse.tile as tile
from concourse import bacc
from concourse.bass_utils import run_bass_kernel_spmd

F32 = mybir.dt.float32
F16 = mybir.dt.float16

B = 2
N = 2048          # sequence length
C = 1024          # model dim
HD = 64           # head dim
SCALE = HD ** -0.5
NT = N // 128     # 16 key tiles
CT = C // 128     # 8 contraction tiles
QC = 4            # q-chunks of 512
QCS = N // QC     # 512
GROUPS = [[0, 1, 2, 3], [4, 5, 6, 7]]
NITER = 8 * NT    # 8 blocks x 16 key tiles

_NC_CACHE = None


def _blk(b):
    """qc-major block order: b = 2*qc + pr."""
    return (b // 2, b % 2)  # (qc, pr)


def build():
    nc = bacc.Bacc(None, target_bir_lowering=False, debug=False)

    # p-major host layouts: every input DMA moves ~128 multi-KB descriptors
    # (HWDGE issue time scales with descriptor count)
    xt_ext = nc.declare_dram_parameter("xt", [QC, 128, CT, QCS], F16, isOutput=False)
    wqk_ext = nc.declare_dram_parameter("wqk", [4, 128, CT, 128], F16, isOutput=False)
    wv_ext = nc.declare_dram_parameter("wv", [128, CT, 256], F16, isOutput=False)
    wpc_ext = nc.declare_dram_parameter("wpc", [128, CT, 256], F16, isOutput=False)
    bc_ext = nc.declare_dram_parameter("bc", [128, 2], F32, isOutput=False)
    out_ext = nc.declare_dram_parameter("out", [256, N], F16, isOutput=True)

    with tile.TileContext(nc) as tc:
        with (
            tc.tile_pool(name="weights", bufs=1) as wpool,
            tc.tile_pool(name="acts", bufs=1) as apool,
            tc.tile_pool(name="expt", bufs=3) as epool,
            tc.tile_pool(name="norm", bufs=2) as npool,
            tc.tile_pool(name="outp", bufs=2) as opool,
            tc.tile_pool(name="ofp", bufs=2) as ofpool,
            tc.tile_pool(name="psS", bufs=2, space="PSUM") as psS_pool,
            tc.tile_pool(name="psE", bufs=2, space="PSUM") as psE_pool,
            tc.tile_pool(name="psO", bufs=1, space="PSUM") as psO_pool,
            tc.tile_pool(name="dramog", bufs=2, space="DRAM") as og_pool,
            tc.tile_pool(name="dramag", bufs=8, space="DRAM") as ag_pool,
        ):
            # ---- SBUF tiles ----
            # xt/wqk chunk-major so each input DMA lands in a contiguous
            # per-partition region (large descriptors)
            xt_sb = apool.tile([128, QC, CT, QCS], F16, tag="xt")
            wqk_sb = wpool.tile([128, 4, CT, 128], F16, tag="wqk")
            wv_sb = wpool.tile([128, CT, 256], F16, tag="wv")
            wp_sb = wpool.tile([128, CT, 256], F16, tag="wp")
            bc_sb = wpool.tile([128, 2], F32, tag="bc")
            ones_row = wpool.tile([1, 64], F16, tag="ones_row")
            qk_sb = apool.tile([128, 4, N], F16, tag="qk")
            v_sb = apool.tile([128, NT, 4, 128], F16, tag="v")

            # constant fills on DVE (a DMA would be thousands of descriptors);
            # v columns 65:128 are never read (PV stationary is 65 cols wide)
            nc.vector.memset(ones_row[:, :], 1.0)
            nc.vector.memset(v_sb[:, :, :, HD:HD + 1], 1.0)

            # preload the Exp activation table while input DMAs stream
            dmy = npool.tile([1, 16], F16, tag="dmy")
            nc.vector.memset(dmy[:, :], 0.0)
            dmy2 = npool.tile([1, 16], F16, tag="dmy2")
            nc.scalar.activation(dmy2[:, :], dmy[:, :],
                                 mybir.ActivationFunctionType.Exp,
                                 bias=0.0, scale=1.0)

            # CC warmup: the first collective pays ~50us of one-time init on
            # the collective cores; run a dummy AllGather now (nothing reads
            # its result) so AG(0,0) executes promptly when issued.
            og_warm = og_pool.tile([1, 16], F16, tag="ogw", name="og_warm")
            nc.sync.dma_start(out=og_warm[:, :], in_=dmy[:, :])
            ag_warm = ag_pool.tile([4, 16], F16, tag="agw", name="ag_warm")
            nc.gpsimd.collective_compute(
                "AllGather", mybir.AluOpType.bypass, replica_groups=GROUPS,
                ins=[og_warm.opt()], outs=[ag_warm.opt()],
            )

            # ---- input DMAs: one per chunk, critical-path first, split over
            # both HWDGE rings (the wqk blocks finish on the ACT ring before
            # the first real exp needs the ACT sequencer).
            # wqk blocks: 0 = q pr0, 1 = q pr1, 2 = k pr0, 3 = k pr1.
            nc.scalar.dma_start(out=wqk_sb[:, 2, :, :], in_=wqk_ext.ap()[2])
            nc.scalar.dma_start(out=wqk_sb[:, 0, :, :], in_=wqk_ext.ap()[0])
            nc.scalar.dma_start(out=wqk_sb[:, 1, :, :], in_=wqk_ext.ap()[1])
            nc.scalar.dma_start(out=wqk_sb[:, 3, :, :], in_=wqk_ext.ap()[3])
            nc.sync.dma_start(out=xt_sb[:, 0, :, :], in_=xt_ext.ap()[0])
            nc.sync.dma_start(out=wv_sb[:, :, :], in_=wv_ext.ap())
            nc.sync.dma_start(out=xt_sb[:, 1, :, :], in_=xt_ext.ap()[1])
            nc.sync.dma_start(out=xt_sb[:, 2, :, :], in_=xt_ext.ap()[2])
            nc.sync.dma_start(out=xt_sb[:, 3, :, :], in_=xt_ext.ap()[3])
            nc.sync.dma_start(out=wp_sb[:, :, :], in_=wpc_ext.ap())
            nc.sync.dma_start(out=bc_sb[:, :], in_=bc_ext[:, :])

            # ---- emission helpers (PE work streamed into the loop) ----
            def xtcol(ct, kt):
                off = (kt % 4) * 128
                return xt_sb[:, kt // 4, ct, off:off + 128]

            def emit_kquad(pr, nch):
                """k for 4 key tiles at once: one stationary per ct streams a
                full 512-col xt chunk, so LDWEIGHTS hides under the matmul."""
                ksl = slice(nch * QCS, (nch + 1) * QCS)
                psq = psE_pool.tile([128, QCS], F32, tag="psE",
                                    name=f"psk_{pr}_{nch}")
                for ct in range(CT):
                    nc.tensor.matmul(
                        psq[:, :],
                        wqk_sb[:, 2 + pr, ct, :],
                        xt_sb[:, nch, ct, :],
                        start=(ct == 0), stop=(ct == CT - 1),
                    )
                nc.vector.tensor_copy(qk_sb[:, 2 + pr, ksl], psq[:, :])

            def emit_khalf(pr, nch, half):
                """k for 2 key tiles (256 cols) — shortens the cold prologue."""
                lo = nch * QCS + half * 256
                psq = psE_pool.tile([128, 256], F32, tag="psE",
                                    name=f"pskh_{pr}_{nch}_{half}")
                for ct in range(CT):
                    nc.tensor.matmul(
                        psq[:, :],
                        wqk_sb[:, 2 + pr, ct, :],
                        xt_sb[:, nch, ct, half * 256:(half + 1) * 256],
                        start=(ct == 0), stop=(ct == CT - 1),
                    )
                nc.vector.tensor_copy(qk_sb[:, 2 + pr, lo:lo + 256], psq[:, :])

            def emit_q(pr, qc):
                qsl = slice(qc * QCS, (qc + 1) * QCS)
                psq = psE_pool.tile([128, QCS], F32, tag="psE", name=f"psq_{pr}_{qc}")
                for ct in range(CT):
                    nc.tensor.matmul(
                        psq[:, :],
                        wqk_sb[:, pr, ct, :],
                        xt_sb[:, qc, ct, :],
                        start=(ct == 0), stop=(ct == CT - 1),
                    )
                nc.vector.tensor_copy(qk_sb[:, pr, qsl], psq[:, :])

            def v_mms(kt):
                """The 8 accumulation matmuls for v(kt), to be interleaved
                between long attention matmuls (hides their LDWEIGHTS)."""
                psv = psE_pool.tile([128, 256], F32, tag="psE", name=f"psv_{kt}")

                def mm(ct, psv=psv, kt=kt):
                    nc.tensor.matmul(
                        psv[:, :],
                        xtcol(ct, kt),
                        wv_sb[:, ct, :],
                        start=(ct == 0), stop=(ct == CT - 1),
                    )

                def fin(psv=psv, kt=kt):
                    nc.vector.tensor_copy(
                        v_sb[:, kt, :, 0:HD],
                        psv[:, :].rearrange("p (h e) -> p h e", h=4),
                    )
                return [lambda ct=ct: mm(ct) for ct in range(CT)], fin

            def scores(b, kt):
                qc, pr = _blk(b)
                qsl = slice(qc * QCS, (qc + 1) * QCS)
                ksl = slice(kt * 128, (kt + 1) * 128)
                psS = psS_pool.tile([128, 2 * QCS], F32, tag="psS",
                                    name=f"psS_{b}_{kt}")
                nc.tensor.matmul(
                    psS[:, 0:QCS],
                    qk_sb[0:64, 2 + pr, ksl],
                    qk_sb[0:64, pr, qsl],
                    start=True, stop=True,
                )
                nc.tensor.matmul(
                    psS[:, QCS:2 * QCS],
                    qk_sb[64:128, 2 + pr, ksl],
                    qk_sb[64:128, pr, qsl],
                    start=True, stop=True,
                )
                return psS

            # ---- per-block normalize / gather / project ----
            norm_state = {}
            ags = {}

            def part1(b):
                """Drain psO (o + rowsum rows) to SBUF; frees psO for the
                next block after just two DVE copies."""
                psO = norm_state.pop(('psO', b))
                o2x = npool.tile([65, QCS], F32, tag="o2x", name=f"o2x_{b}")
                o2y = npool.tile([65, QCS], F32, tag="o2y", name=f"o2y_{b}")
                nc.vector.tensor_copy(o2x[:, :], psO[0:65, 0:QCS])
                nc.vector.tensor_copy(o2y[:, :], psO[0:65, QCS:2 * QCS])
                norm_state[('o2', b)] = (o2x, o2y)

            def part2(b):
                """1/rowsum (fast approx), broadcast via PE matmul, normalize."""
                o2x, o2y = norm_state.pop(('o2', b))
                psB = psE_pool.tile([128, QCS], F32, tag="psE", name=f"psB_{b}")
                for hh, o2 in ((0, o2x), (1, o2y)):
                    rs = npool.tile([1, QCS], F32, tag="rs", name=f"rs_{b}_{hh}")
                    nc.vector.tensor_copy(rs[:, :], o2[64:65, :])
                    rcf = npool.tile([1, QCS], F32, tag="rcf", name=f"rcf_{b}_{hh}")
                    nc.vector.reciprocal_approx_fast(out=rcf[:, :], in_=rs[:, :])
                    rc16 = npool.tile([1, QCS], F16, tag="rc16", name=f"rc16_{b}_{hh}")
                    nc.vector.tensor_copy(rc16[:, :], rcf[:, :])
                    nc.tensor.matmul(psB[hh * 64:(hh + 1) * 64, :],
                                     ones_row[:, :], rc16[:, :],
                                     start=True, stop=True)
                on_sb = npool.tile([128, QCS], F16, tag="on", name=f"on_{b}")
                nc.vector.tensor_mul(on_sb[0:64, :], o2x[0:64, :], psB[0:64, :])
                nc.vector.tensor_mul(on_sb[64:128, :], o2y[0:64, :],
                                     psB[64:128, :])
                norm_state[('on', b)] = on_sb

            def part3(b):
                """Store this block's head outputs; one AllGather per qc
                (both pr halves together) to halve the serial CC chain."""
                qc, pr = _blk(b)
                on_sb = norm_state.pop(('on', b))
                if pr == 0:
                    og = og_pool.tile([256, QCS], F16, tag="og", name=f"og_{qc}")
                    norm_state[('og', qc)] = og
                else:
                    og = norm_state.pop(('og', qc))
                nc.sync.dma_start(out=og[pr * 128:(pr + 1) * 128, :],
                                  in_=on_sb[:, :])
                if pr == 1:
                    ag = ag_pool.tile([1024, QCS], F16, tag="ag", name=f"ag_{qc}")
                    nc.gpsimd.collective_compute(
                        "AllGather",
                        mybir.AluOpType.bypass,
                        replica_groups=GROUPS,
                        ins=[og.opt()],
                        outs=[ag.opt()],
                    )
                    ags[qc] = ag

            of_sbs = {}

            def emit_ofload(qc, split_rings=False):
                of_sb = ofpool.tile([128, CT, QCS], F16, tag="of",
                                    name=f"of_{qc}")
                of_sbs[qc] = of_sb
                ag_r = ags[qc][:, :].rearrange("(t p) n -> p t n", p=128)
                if split_rings:
                    nc.sync.dma_start(out=of_sb[:, 0:4, :], in_=ag_r[:, 0:4, :])
                    nc.scalar.dma_start(out=of_sb[:, 4:8, :], in_=ag_r[:, 4:8, :])
                else:
                    nc.sync.dma_start(out=of_sb[:, :, :], in_=ag_r)

            def emit_proj(qc, m2):
                qsl = slice(qc * QCS, (qc + 1) * QCS)
                of_sb = of_sbs[qc]
                psP = psE_pool.tile([128, QCS], F32, tag="psE",
                                    name=f"psP_{qc}_{m2}")
                for t in range(CT):
                    nc.tensor.matmul(
                        psP[:, :],
                        wp_sb[:, t, m2 * 128:(m2 + 1) * 128],
                        of_sb[:, t, :],
                        start=(t == 0), stop=(t == CT - 1),
                    )
                outsb = opool.tile([128, QCS], F16, tag="outsb",
                                   name=f"outsb_{qc}_{m2}")
                nc.vector.tensor_scalar_add(outsb[:, :], psP[:, :],
                                            bc_sb[:, m2:m2 + 1])
                nc.sync.dma_start(out=out_ext[m2 * 128:(m2 + 1) * 128, qsl],
                                  in_=outsb[:, :])

            # ---- static emission schedule: iter -> list of thunks ----
            sched = {}

            def at(i, fn):
                sched.setdefault(i, []).append(fn)

            at(0, lambda: emit_khalf(0, 0, 1))    # k(pr0) kt2-3, due iter 1
            at(1, lambda: emit_kquad(0, 1))       # k(pr0) due iters 4/8/12
            at(5, lambda: emit_kquad(0, 2))
            at(9, lambda: emit_kquad(0, 3))
            at(11, lambda: emit_q(1, 0))          # due block 1 (iter 15)
            at(12, lambda: emit_kquad(1, 0))      # k(pr1) due iter 16/20/24/28
            at(17, lambda: emit_kquad(1, 1))
            at(21, lambda: emit_kquad(1, 2))
            at(25, lambda: emit_kquad(1, 3))
            at(27, lambda: emit_q(0, 1))          # due block 2 (iter 31)
            at(43, lambda: emit_q(1, 1))          # due block 3 (iter 47)
            at(59, lambda: emit_q(0, 2))          # due block 4 (iter 63)
            at(75, lambda: emit_q(1, 2))          # due block 5 (iter 79)
            at(91, lambda: emit_q(0, 3))          # due block 6 (iter 95)
            at(107, lambda: emit_q(1, 3))         # due block 7 (iter 111)
            # proj(qc): AG(qc) is issued at iter 32qc+34; leave ~2 blocks of
            # slack so core-launch skew + the serial CC chain never stall PE
            at(66, lambda: emit_ofload(0))
            at(68, lambda: emit_proj(0, 0))
            at(70, lambda: emit_proj(0, 1))
            at(90, lambda: emit_ofload(1))
            at(92, lambda: emit_proj(1, 0))
            at(94, lambda: emit_proj(1, 1))
            at(114, lambda: emit_ofload(2))
            at(116, lambda: emit_proj(2, 0))
            at(118, lambda: emit_proj(2, 1))
            for b in range(7):                    # normalize tail of each block
                at(16 * b + 17, lambda b=b: part2(b))
                at(16 * b + 18, lambda b=b: part3(b))

            # v(kt) streamed during block 0, matmuls interleaved between the
            # long attention matmuls so each LDWEIGHTS hides under them
            v_plan = {kt - 1: kt for kt in range(1, NT)}

            # ---- prologue (kept short: it runs at the cold PE clock; v(0)
            # comes after the first scores so exp(0) starts sooner) ----
            emit_khalf(0, 0, 0)
            emit_q(0, 0)
            psS_cur = scores(0, 0)
            vm, vfin = v_mms(0)
            for m in vm:
                m()
            vfin()

            # ---- main loop ----
            for bi in range(NITER):
                b, kt = bi // NT, bi % NT
                qc, pr = _blk(b)
                vkt = v_plan.get(bi)
                vm, vfin = v_mms(vkt) if vkt is not None else ([], None)
                if bi + 1 < NITER:
                    nb, nkt = (b, kt + 1) if kt < NT - 1 else (b + 1, 0)
                    nqc, npr = _blk(nb)
                    qsl_ = slice(nqc * QCS, (nqc + 1) * QCS)
                    ksl_ = slice(nkt * 128, (nkt + 1) * 128)
                    psS_next = psS_pool.tile([128, 2 * QCS], F32, tag="psS",
                                             name=f"psS_{nb}_{nkt}")
                    nc.tensor.matmul(psS_next[:, 0:QCS],
                                     qk_sb[0:64, 2 + npr, ksl_],
                                     qk_sb[0:64, npr, qsl_],
                                     start=True, stop=True)
                    for m in vm[0:2]:
                        m()
                    nc.tensor.matmul(psS_next[:, QCS:2 * QCS],
                                     qk_sb[64:128, 2 + npr, ksl_],
                                     qk_sb[64:128, npr, qsl_],
                                     start=True, stop=True)
                    for m in vm[2:4]:
                        m()
                else:
                    psS_next = None
                    for m in vm[0:4]:
                        m()
                expt = epool.tile([128, 2 * QCS], F16, tag="expt",
                                  name=f"expt_{bi}")
                nc.scalar.activation(
                    expt[:, :], psS_cur[:, :],
                    mybir.ActivationFunctionType.Exp,
                    bias=0.0, scale=SCALE,
                )
                for fn in sched.get(bi, ()):
                    fn()
                if kt == 0:
                    norm_state[('psO', b)] = psO_pool.tile(
                        [128, 2 * QCS], F32, tag="psO", name=f"psO_{b}")
                psO = norm_state[('psO', b)]
                nc.tensor.matmul(
                    psO[0:65, 0:QCS],
                    v_sb[:, kt, 2 * pr, 0:65],
                    expt[:, 0:QCS],
                    start=(kt == 0), stop=(kt == NT - 1),
                )
                for m in vm[4:6]:
                    m()
                nc.tensor.matmul(
                    psO[0:65, QCS:2 * QCS],
                    v_sb[:, kt, 2 * pr + 1, 0:65],
                    expt[:, QCS:2 * QCS],
                    start=(kt == 0), stop=(kt == NT - 1),
                )
                for m in vm[6:8]:
                    m()
                if vfin is not None:
                    vfin()
                psS_cur = psS_next
                if kt == NT - 1:
                    part1(b)

            # ---- tail: last block's normalize + gather + projection ----
            part2(7)
            part3(7)
            emit_ofload(3, split_rings=True)
            emit_proj(3, 0)
            emit_proj(3, 1)

    nc.compile()
    return nc


def _get_nc():
    global _NC_CACHE
    if _NC_CACHE is None:
        _NC_CACHE = build()
    return _NC_CACHE


def shard_inputs(x, w_qkv, w_proj, b_proj):
    x = np.asarray(x, dtype=np.float32)
    w_qkv = np.asarray(w_qkv, dtype=np.float32)
    w_proj = np.asarray(w_proj, dtype=np.float32)
    b_proj = np.asarray(b_proj, dtype=np.float32)
    # merged-AG row order: rank-major, then pr, then local head:
    # rows [j*256 + pr*128 + h2*64 + e] <-> global head 4j + 2*pr + h2
    perm = np.arange(1024).reshape(16, 64)[
        [4 * j + 2 * pr + h2
         for j in range(4) for pr in range(2) for h2 in range(2)]
    ].reshape(-1)
    def pmajor_kt(w):
        # [C, M] -> [128(p), CT(t), M]: row t*128+p -> [p, t]
        return np.ascontiguousarray(
            w.reshape(CT, 128, w.shape[1]).transpose(1, 0, 2).astype(np.float16))

    in_maps = []
    for core in range(8):
        b, g = divmod(core, 4)
        cs = slice(g * 256, (g + 1) * 256)
        xtT = x[b].T  # [C, N]
        # xt: [QC(nch), 128(p), CT(t), QCS] with [nch,p,t,c] = xtT[t*128+p, nch*512+c]
        xt_arr = np.ascontiguousarray(
            xtT.reshape(CT, 128, QC, QCS).transpose(2, 1, 0, 3).astype(np.float16))
        # wqk blocks: 0 = q pr0, 1 = q pr1, 2 = k pr0, 3 = k pr1 (128 cols each)
        qcols = w_qkv[:, 0 * C + g * 256:0 * C + (g + 1) * 256]
        kcols = w_qkv[:, 1 * C + g * 256:1 * C + (g + 1) * 256]
        wqk = np.concatenate([qcols, kcols], axis=1)  # [C, 512]
        wqk_arr = np.ascontiguousarray(
            wqk.reshape(CT, 128, 4, 128).transpose(2, 1, 0, 3).astype(np.float16))
        in_maps.append({
            "xt": xt_arr,
            "wqk": wqk_arr,
            "wv": pmajor_kt(w_qkv[:, 2 * C + g * 256:2 * C + (g + 1) * 256]),
            "wpc": pmajor_kt(w_proj[perm, :][:, cs]),
            "bc": np.ascontiguousarray(b_proj[cs].reshape(2, 128).T),
        })
    return in_maps


def assemble_output(results):
    outT = np.empty((B, C, N), dtype=np.float32)
    for core in range(8):
        b, g = divmod(core, 4)
        outT[b, g * 256:(g + 1) * 256, :] = np.asarray(results[core]["out"], dtype=np.float32)
    return np.ascontiguousarray(outT.transpose(0, 2, 1))


def run_sharded(x, w_qkv, w_proj, b_proj, trace=False):
    nc = _get_nc()
    in_maps = shard_inputs(x, w_qkv, w_proj, b_proj)
    res = run_bass_kernel_spmd(nc, in_maps, core_ids=list(range(8)), trace=trace)
    return assemble_output(res.results), res.exec_time_ns


def kernel(x, w_qkv, w_proj, b_proj):
    out, _ = run_sharded(x, w_qkv, w_proj, b_proj, trace=False)
    return out
